# revision 4
# baseline (speedup 1.0000x reference)
"""NerfExperts MoE kernel for Trainium2, expert-parallel over 8 NeuronCores.

Strategy: each of the 1024 points is routed to one of 100 experts
(~2.3MB of fp32 weights each, ~232MB total -> memory bound).  We shard
the *experts* across the 8 cores (13 slots per core), dispatch tokens to
their expert's core on the host, and stream each expert's weights from
HBM exactly once.  Activations stay transposed ([feature, token]) so
every layer is a plain stationary-weight matmul with no transposes.
"""

import numpy as np

import concourse.bass as bass
import concourse.bacc as bacc
import concourse.mybir as mybir
import concourse.tile as tile
from concourse.bass_utils import run_bass_kernel_spmd

PI = float(np.pi)
N_CORES = 8
E = 100
H = 256
HD = 128
NX, ND = 6, 4
CAP_MAX = 128  # max tokens per expert slot (keeps matmul N and PSUM in range)

# ---------------------------------------------------------------------------
# Weight blob column layout (fp32 columns within a [128, NCOLS] per-expert
# blob).  Row convention for embedding-consuming slabs: sin rows at
# partitions 0:18 (0:12 for dirs), cos at 32:50 (32:44), xyz at 64:67,
# dead rows zeroed.
# ---------------------------------------------------------------------------
L0_W, L0_B = 0, 256                      # [0:256) w0 slab, [256:258) b0
_off = 258
MID_OFF = {}
for _l in (1, 2, 3, 4):
    MID_OFF[_l] = _off
    _off += 514                          # 2 k-slabs x 256 + 2 bias cols
L5_OFF = _off
_off += 770                              # A(256) B(256) skip(256) + 2 bias
for _l in (6, 7):
    MID_OFF[_l] = _off
    _off += 514
WI_OFF = _off
_off += 514
WA_OFF = _off
_off += 3                                # 2 weight cols + 1 bias col
WC0_OFF = _off
_off += 385                              # A(128) B(128) rays(128) + 1 bias
WC1_OFF = _off
_off += 4                                # 3 weight cols + 1 bias col
NCOLS = _off                             # 5018


def _pack_expert_blob(blob, inputs, e):
    """Fill one [128, NCOLS] fp32 blob for expert e."""
    w0 = inputs["w0"][e]                             # [39, 256]
    blob[0:18, 0:256] = w0[0:18]
    blob[32:50, 0:256] = w0[18:36]
    blob[64:67, 0:256] = w0[36:39]
    b0 = inputs["b0"][e]
    blob[:, 256] = b0[0:128]
    blob[:, 257] = b0[128:256]
    for l in (1, 2, 3, 4, 6, 7):
        w = inputs[f"w{l}"][e]                       # [256, 256]
        b = inputs[f"b{l}"][e]
        o = MID_OFF[l]
        for k in (0, 1):
            blob[:, o + k * 256: o + (k + 1) * 256] = w[128 * k: 128 * (k + 1)]
        blob[:, o + 512] = b[0:128]
        blob[:, o + 513] = b[128:256]
    w5 = inputs["w5"][e]                             # [295, 256]
    b5 = inputs["b5"][e]
    for k in (0, 1):
        blob[:, L5_OFF + k * 256: L5_OFF + (k + 1) * 256] = w5[128 * k: 128 * (k + 1)]
    blob[0:18, L5_OFF + 512: L5_OFF + 768] = w5[256:274]
    blob[32:50, L5_OFF + 512: L5_OFF + 768] = w5[274:292]
    blob[64:67, L5_OFF + 512: L5_OFF + 768] = w5[292:295]
    blob[:, L5_OFF + 768] = b5[0:128]
    blob[:, L5_OFF + 769] = b5[128:256]
    wi = inputs["wi"][e]
    bi = inputs["bi"][e]
    for k in (0, 1):
        blob[:, WI_OFF + k * 256: WI_OFF + (k + 1) * 256] = wi[128 * k: 128 * (k + 1)]
    blob[:, WI_OFF + 512] = bi[0:128]
    blob[:, WI_OFF + 513] = bi[128:256]
    wa = inputs["wa"][e][:, 0]                       # [256]
    blob[:, WA_OFF] = wa[0:128]
    blob[:, WA_OFF + 1] = wa[128:256]
    blob[0, WA_OFF + 2] = inputs["ba"][e][0]
    wc0 = inputs["wc0"][e]                           # [283, 128]
    blob[:, WC0_OFF: WC0_OFF + 128] = wc0[0:128]
    blob[:, WC0_OFF + 128: WC0_OFF + 256] = wc0[128:256]
    blob[0:12, WC0_OFF + 256: WC0_OFF + 384] = wc0[256:268]
    blob[32:44, WC0_OFF + 256: WC0_OFF + 384] = wc0[268:280]
    blob[64:67, WC0_OFF + 256: WC0_OFF + 384] = wc0[280:283]
    blob[:, WC0_OFF + 384] = inputs["bc0"][e]
    wc1 = inputs["wc1"][e]                           # [128, 3]
    blob[:, WC1_OFF: WC1_OFF + 3] = wc1
    blob[0:3, WC1_OFF + 3] = inputs["bc1"][e]


# ---------------------------------------------------------------------------
# Device program
# ---------------------------------------------------------------------------

def _build_program(C, nslot, group=2, wbufs=4):
    """Build the SPMD Bass program: nslot expert slots of C tokens each."""
    nall = nslot * C
    dt = mybir.dt.float32
    f32 = mybir.dt.float32
    Sin = mybir.ActivationFunctionType.Sin
    Sigmoid = mybir.ActivationFunctionType.Sigmoid
    ADD = mybir.AluOpType.add
    SUB = mybir.AluOpType.subtract
    MUL = mybir.AluOpType.mult
    MAX = mybir.AluOpType.max
    MIN = mybir.AluOpType.min
    # range-reduction constants (Cody-Waite, fp32 magic rounding)
    INV2PI = float(np.float32(1.0 / (2 * PI)))
    MAGIC = 12582912.0            # 1.5 * 2**23: forces round-to-int in fp32
    C1 = 6.28125                  # 2*pi high part, exact in fp32
    C2 = float(np.float32(2 * PI - 6.28125))
    CLAMP = 3.1415925             # just under pi (ACT Sin domain is [-pi, pi])
    HALF_PI = float(np.float32(PI / 2))

    nc = bacc.Bacc("TRN2", target_bir_lowering=False, debug=False)
    wt_d = nc.dram_tensor("wt", (nslot, 128, NCOLS), dt, kind="ExternalInput")
    pts_d = nc.dram_tensor("ptsT", (3, nall), dt, kind="ExternalInput")
    dir_d = nc.dram_tensor("dirT", (3, nall), dt, kind="ExternalInput")
    fx_d = nc.dram_tensor("fx", (3, 18), dt, kind="ExternalInput")
    fd_d = nc.dram_tensor("fd", (3, 12), dt, kind="ExternalInput")
    al_d = nc.dram_tensor("alpha_out", (1, nall), dt, kind="ExternalOutput")
    co_d = nc.dram_tensor("color_out", (3, nall), dt, kind="ExternalOutput")

    with tile.TileContext(nc) as tc:
        with (
            tc.tile_pool(name="wp", bufs=wbufs) as wp,
            tc.tile_pool(name="cp", bufs=1) as cp,
            tc.tile_pool(name="xp", bufs=3 * group + 2) as xp,
            tc.tile_pool(name="psA", bufs=6, space=bass.MemorySpace.PSUM) as psA,
            tc.tile_pool(name="psB", bufs=2, space=bass.MemorySpace.PSUM) as psB,
        ):
            # ---- constants + embeddings (once per core) ----
            embP = cp.tile([67, nall], f32)   # points: sin 0:18, cos 32:50, xyz 64:67
            embD = cp.tile([67, nall], f32)   # dirs:   sin 0:12, cos 32:44, xyz 64:67
            nc.vector.memset(embP[:], 0.0)
            nc.vector.memset(embD[:], 0.0)
            fx_sb = cp.tile([3, 18], f32)
            nc.sync.dma_start(fx_sb[:], fx_d.ap()[:])
            fd_sb = cp.tile([3, 12], f32)
            nc.sync.dma_start(fd_sb[:], fd_d.ap()[:])
            pts_sb = cp.tile([3, nall], f32)
            nc.sync.dma_start(pts_sb[:], pts_d.ap()[:])
            dir_sb = cp.tile([3, nall], f32)
            nc.sync.dma_start(dir_sb[:], dir_d.ap()[:])
            nc.vector.tensor_copy(embP[64:67, :], pts_sb[:])
            nc.vector.tensor_copy(embD[64:67, :], dir_sb[:])

            alpha_sb = cp.tile([1, nall], f32)
            color_sb = cp.tile([3, nall], f32)

            # frequency expansion + range-reduced sin/cos, in <=512-col chunks.
            # phase t lives in psum; v = t - round(t/2pi)*2pi via magic-constant
            # rounding + two-step Cody-Waite, clamped into ACT Sin's domain.
            def reduce_sin(tsrc, rows, ncol):
                t1 = xp.tile([rows, ncol], f32, tag="vred")
                nc.vector.tensor_scalar(t1[:], tsrc, INV2PI, MAGIC, MUL, ADD)
                r = xp.tile([rows, ncol], f32, tag="vred")
                nc.vector.tensor_scalar(r[:], t1[:], MAGIC, None, SUB)
                rd = xp.tile([rows, ncol], f32, tag="vred")
                nc.vector.scalar_tensor_tensor(rd[:], r[:], -C1, tsrc, MUL, ADD)
                rd2 = xp.tile([rows, ncol], f32, tag="vred")
                nc.vector.scalar_tensor_tensor(rd2[:], r[:], -C2, rd[:], MUL, ADD)
                v = xp.tile([rows, ncol], f32, tag="vred")
                nc.vector.tensor_scalar(v[:], rd2[:], CLAMP, -CLAMP, MIN, MAX)
                return v

            for lo in range(0, nall, 512):
                hi = min(nall, lo + 512)
                w_ = hi - lo
                for (rows, fmat, src, dst) in (
                    (18, fx_sb, pts_sb, embP),
                    (12, fd_sb, dir_sb, embD),
                ):
                    ep = psA.tile([rows, w_], f32, tag="mlp")
                    nc.tensor.matmul(ep[:], fmat[:, 0:rows], src[:, lo:hi],
                                     start=True, stop=True)
                    vs = reduce_sin(ep[:], rows, w_)
                    nc.scalar.activation(dst[0:rows, lo:hi], vs[:], Sin)
                    pre = xp.tile([rows, w_], f32, tag="vred")
                    nc.vector.tensor_scalar(pre[:], ep[:], HALF_PI, None, ADD)
                    vc = reduce_sin(pre[:], rows, w_)
                    cs = xp.tile([rows, w_], f32, tag="vred")
                    nc.scalar.activation(cs[:], vc[:], Sin)
                    nc.vector.tensor_copy(dst[32:32 + rows, lo:hi], cs[:])

            # ---- per-expert-slot MLP, experts interleaved in groups ----
            def stage_mid(w, xin, off, sl):
                """One 256->256 layer: returns psum tile [128, 2C]."""
                ps = psA.tile([128, 2 * C], f32, tag="mlp")
                for j in (0, 1):
                    nc.tensor.matmul(ps[:, j * C:(j + 1) * C],
                                     w[0:128, off + j * 128: off + j * 128 + 128],
                                     xin[:, 0, :], start=True, stop=False)
                    nc.tensor.matmul(ps[:, j * C:(j + 1) * C],
                                     w[0:128, off + 256 + j * 128: off + 256 + j * 128 + 128],
                                     xin[:, 1, :], start=False, stop=True)
                return ps

            def relu_out(w, ps, boff):
                xn = xp.tile([128, 2, C], f32, tag="x")
                for j in (0, 1):
                    nc.vector.tensor_scalar(
                        xn[:, j, :], ps[:, j * C:(j + 1) * C],
                        w[0:128, boff + j: boff + j + 1], 0.0, ADD, MAX)
                return xn

            class SlotState:
                pass

            states = [SlotState() for _ in range(nslot)]

            def emit_stage(s, st, stage):
                w = st.w
                sl = slice(s * C, (s + 1) * C)
                if stage == 0:  # L0
                    ps = psA.tile([128, 2 * C], f32, tag="mlp")
                    for j in (0, 1):
                        nc.tensor.matmul(ps[:, j * C:(j + 1) * C],
                                         w[0:67, j * 128: j * 128 + 128],
                                         embP[0:67, sl], start=True, stop=True)
                    st.x = relu_out(w, ps, L0_B)
                elif stage in (1, 2, 3, 4, 6, 7):  # mid layers
                    o = MID_OFF[stage]
                    ps = stage_mid(w, st.x, o, s)
                    st.x = relu_out(w, ps, o + 512)
                elif stage == 5:  # skip layer
                    ps = psA.tile([128, 2 * C], f32, tag="mlp")
                    for j in (0, 1):
                        pj = ps[:, j * C:(j + 1) * C]
                        nc.tensor.matmul(pj, w[0:128, L5_OFF + j * 128: L5_OFF + j * 128 + 128],
                                         st.x[:, 0, :], start=True, stop=False)
                        nc.tensor.matmul(pj, w[0:128, L5_OFF + 256 + j * 128: L5_OFF + 256 + j * 128 + 128],
                                         st.x[:, 1, :], start=False, stop=False)
                        nc.tensor.matmul(pj, w[0:67, L5_OFF + 512 + j * 128: L5_OFF + 512 + j * 128 + 128],
                                         embP[0:67, sl], start=False, stop=True)
                    st.x = relu_out(w, ps, L5_OFF + 768)
                elif stage == 8:  # wi -> inter (bias, no relu)
                    ps = stage_mid(w, st.x, WI_OFF, s)
                    it = xp.tile([128, 2, C], f32, tag="x")
                    for j in (0, 1):
                        nc.vector.tensor_scalar(
                            it[:, j, :], ps[:, j * C:(j + 1) * C],
                            w[0:128, WI_OFF + 512 + j: WI_OFF + 513 + j], None, ADD)
                    st.it = it
                elif stage == 9:  # wa -> alpha
                    pa = psB.tile([3, C], f32, tag="head")
                    nc.tensor.matmul(pa[0:1, :], w[0:128, WA_OFF: WA_OFF + 1],
                                     st.x[:, 0, :], start=True, stop=False)
                    nc.tensor.matmul(pa[0:1, :], w[0:128, WA_OFF + 1: WA_OFF + 2],
                                     st.x[:, 1, :], start=False, stop=True)
                    nc.vector.tensor_scalar(alpha_sb[0:1, sl], pa[0:1, :],
                                            w[0:1, WA_OFF + 2: WA_OFF + 3], None, ADD)
                elif stage == 10:  # wc0 -> c (relu)
                    pc = psA.tile([128, C], f32, tag="mlp")
                    nc.tensor.matmul(pc[:], w[0:128, WC0_OFF: WC0_OFF + 128],
                                     st.it[:, 0, :], start=True, stop=False)
                    nc.tensor.matmul(pc[:], w[0:128, WC0_OFF + 128: WC0_OFF + 256],
                                     st.it[:, 1, :], start=False, stop=False)
                    nc.tensor.matmul(pc[:], w[0:67, WC0_OFF + 256: WC0_OFF + 384],
                                     embD[0:67, sl], start=False, stop=True)
                    ct = xp.tile([128, C], f32, tag="ct")
                    nc.vector.tensor_scalar(ct[:], pc[:],
                                            w[0:128, WC0_OFF + 384: WC0_OFF + 385],
                                            0.0, ADD, MAX)
                    st.ct = ct
                elif stage == 11:  # wc1 -> sigmoid color
                    pcol = psB.tile([3, C], f32, tag="head")
                    nc.tensor.matmul(pcol[:], w[0:128, WC1_OFF: WC1_OFF + 3],
                                     st.ct[:], start=True, stop=True)
                    nc.scalar.activation(color_sb[0:3, sl], pcol[:], Sigmoid,
                                         bias=w[0:3, WC1_OFF + 3: WC1_OFF + 4])

            for g0 in range(0, nslot, group):
                slots = range(g0, min(g0 + group, nslot))
                for s in slots:
                    st = states[s]
                    st.w = wp.tile([128, NCOLS], f32, tag="w")
                    nc.sync.dma_start(st.w[:], wt_d.ap()[s])
                for stage in range(12):
                    for s in slots:
                        emit_stage(s, states[s], stage)

            nc.sync.dma_start(al_d.ap()[:], alpha_sb[:])
            nc.sync.dma_start(co_d.ap()[:], color_sb[:])

    nc.compile()
    return nc


_prog_cache = {}
_last_results = None


def _get_program(C, nslot):
    key = (C, nslot)
    if key not in _prog_cache:
        _prog_cache[key] = _build_program(C, nslot)
    return _prog_cache[key]


# ---------------------------------------------------------------------------
# Host wrapper
# ---------------------------------------------------------------------------

def kernel(**inputs):
    global _last_results
    inputs = {k: np.asarray(v) for k, v in inputs.items()}
    idx = inputs["index"].astype(np.int64)
    B = idx.shape[0]
    points = inputs["points"].astype(np.float32)
    dirs = inputs["directions"].astype(np.float32)

    # --- routing: split each expert's tokens into <=CAP_MAX chunks ("virtual
    # experts"), distribute round-robin (sorted by size) over 8 cores ---
    tok = [np.nonzero(idx == e)[0] for e in range(E)]
    virt = []  # (expert, token_ids)
    for e in range(E):
        t = tok[e]
        if len(t) == 0:
            continue
        for lo in range(0, len(t), CAP_MAX):
            virt.append((e, t[lo: lo + CAP_MAX]))
    if not virt:
        virt = [(0, np.zeros((0,), np.int64))]
    virt.sort(key=lambda v: -len(v[1]))
    nslot = max(1, int(np.ceil(len(virt) / N_CORES)))
    C = max(4, int(np.ceil(max(len(v[1]) for v in virt) / 4) * 4))
    nall = nslot * C

    core_slots = [[] for _ in range(N_CORES)]
    for i, v in enumerate(virt):
        core_slots[i % N_CORES].append(v)

    nc = _get_program(C, nslot)

    fx = np.zeros((3, 18), np.float32)
    for c in range(3):
        for k in range(NX):
            fx[c, c * NX + k] = float(2 ** k)
    fd = np.zeros((3, 12), np.float32)
    for c in range(3):
        for k in range(ND):
            fd[c, c * ND + k] = float(2 ** k)

    in_maps = []
    for c in range(N_CORES):
        wt = np.zeros((nslot, 128, NCOLS), np.float32)
        ptsT = np.zeros((3, nall), np.float32)
        dirT = np.zeros((3, nall), np.float32)
        for s, (e, t) in enumerate(core_slots[c]):
            _pack_expert_blob(wt[s], inputs, e)
            n = len(t)
            if n:
                ptsT[:, s * C: s * C + n] = points[t].T
                dirT[:, s * C: s * C + n] = dirs[t].T
        in_maps.append({"wt": wt, "ptsT": ptsT, "dirT": dirT, "fx": fx, "fd": fd})

    res = run_bass_kernel_spmd(nc, in_maps, core_ids=list(range(N_CORES)))
    _last_results = res

    out = np.zeros((B, 4), np.float32)
    for c in range(N_CORES):
        al = res.results[c]["alpha_out"]
        co = res.results[c]["color_out"]
        for s, (e, t) in enumerate(core_slots[c]):
            n = len(t)
            if n:
                out[t, 0] = al[0, s * C: s * C + n]
                out[t, 1:4] = co[:, s * C: s * C + n].T
    return out


# revision 8
# speedup vs baseline: 2.9031x; 2.9031x over previous
"""NerfExperts MoE kernel for Trainium2, expert-parallel over 8 NeuronCores.

Strategy: each of the 1024 points is routed to one of 100 experts
(~2.3MB of fp32 weights each, ~232MB total -> memory bound).  We shard
the *experts* across the 8 cores (13 slots per core), dispatch tokens to
their expert's core on the host, and stream each expert's weights from
HBM exactly once, as bf16 (halves traffic, halves PE weight-load time).
Activations stay transposed ([feature, token]) so every layer is a plain
stationary-weight matmul with no transposes.  Matmul accumulation is
fp32 in PSUM; biases are applied in fp32.  Harmonic-embedding phases are
computed in fp32 with Cody-Waite range reduction for ACT's Sin.
"""

import numpy as np
import ml_dtypes

import concourse.bass as bass
import concourse.bacc as bacc
import concourse.mybir as mybir
import concourse.tile as tile
from concourse.bass_utils import run_bass_kernel_spmd

PI = float(np.pi)
N_CORES = 8
E = 100
NX, ND = 6, 4
CAP_MAX = 128  # max tokens per expert slot (keeps matmul N and PSUM in range)

# ---------------------------------------------------------------------------
# Weight blob column layout (bf16 columns within a [128, NCOLS] per-expert
# blob).  Row convention for embedding-consuming slabs: sin rows at
# partitions 0:18 (0:12 for dirs), cos at 32:50 (32:44), xyz at 64:67,
# dead rows zeroed.  Bias columns live in a separate fp32 tensor.
# ---------------------------------------------------------------------------
L0_W = 0                                 # [0:256) w0 slab
_off = 256
MID_OFF = {}
for _l in (1, 2, 3, 4):
    MID_OFF[_l] = _off
    _off += 512                          # 2 k-slabs x 256
L5_OFF = _off
_off += 768                              # A(256) B(256) skip(256)
for _l in (6, 7):
    MID_OFF[_l] = _off
    _off += 512
WI_OFF = _off
_off += 512
WA_OFF = _off
_off += 2                                # 2 weight cols
WC0_OFF = _off
_off += 384                              # A(128) B(128) rays(128)
WC1_OFF = _off
_off += 3                                # 3 weight cols
NCOLS = _off                             # 4997

# fp32 bias tensor column map: [*, 128, NB]
BIAS_COL = {}
BIAS_COL[0] = 0                          # L0: cols 0,1
_b = 2
for _l in (1, 2, 3, 4, 6, 7):
    BIAS_COL[_l] = _b
    _b += 2
BIAS_COL[5] = _b
_b += 2
BIAS_COL["wi"] = _b
_b += 2
BIAS_COL["ba"] = _b
_b += 1
BIAS_COL["bc0"] = _b
_b += 1
BIAS_COL["bc1"] = _b
_b += 1
NB = _b                                  # 21


def _pack_expert(blob, bt, inputs, e):
    """Fill one [128, NCOLS] bf16 blob + [128, NB] fp32 bias slab."""
    w0 = inputs["w0"][e]                             # [39, 256]
    blob[0:18, 0:256] = w0[0:18]
    blob[32:50, 0:256] = w0[18:36]
    blob[64:67, 0:256] = w0[36:39]
    b0 = inputs["b0"][e]
    bt[:, 0] = b0[0:128]
    bt[:, 1] = b0[128:256]
    for l in (1, 2, 3, 4, 6, 7):
        w = inputs[f"w{l}"][e]                       # [256, 256]
        b = inputs[f"b{l}"][e]
        o = MID_OFF[l]
        for k in (0, 1):
            blob[:, o + k * 256: o + (k + 1) * 256] = w[128 * k: 128 * (k + 1)]
        bt[:, BIAS_COL[l]] = b[0:128]
        bt[:, BIAS_COL[l] + 1] = b[128:256]
    w5 = inputs["w5"][e]                             # [295, 256]
    b5 = inputs["b5"][e]
    for k in (0, 1):
        blob[:, L5_OFF + k * 256: L5_OFF + (k + 1) * 256] = w5[128 * k: 128 * (k + 1)]
    blob[0:18, L5_OFF + 512: L5_OFF + 768] = w5[256:274]
    blob[32:50, L5_OFF + 512: L5_OFF + 768] = w5[274:292]
    blob[64:67, L5_OFF + 512: L5_OFF + 768] = w5[292:295]
    bt[:, BIAS_COL[5]] = b5[0:128]
    bt[:, BIAS_COL[5] + 1] = b5[128:256]
    wi = inputs["wi"][e]
    for k in (0, 1):
        blob[:, WI_OFF + k * 256: WI_OFF + (k + 1) * 256] = wi[128 * k: 128 * (k + 1)]
    bt[:, BIAS_COL["wi"]] = inputs["bi"][e][0:128]
    bt[:, BIAS_COL["wi"] + 1] = inputs["bi"][e][128:256]
    wa = inputs["wa"][e][:, 0]                       # [256]
    blob[:, WA_OFF] = wa[0:128]
    blob[:, WA_OFF + 1] = wa[128:256]
    bt[0, BIAS_COL["ba"]] = inputs["ba"][e][0]
    wc0 = inputs["wc0"][e]                           # [283, 128]
    blob[:, WC0_OFF: WC0_OFF + 128] = wc0[0:128]
    blob[:, WC0_OFF + 128: WC0_OFF + 256] = wc0[128:256]
    blob[0:12, WC0_OFF + 256: WC0_OFF + 384] = wc0[256:268]
    blob[32:44, WC0_OFF + 256: WC0_OFF + 384] = wc0[268:280]
    blob[64:67, WC0_OFF + 256: WC0_OFF + 384] = wc0[280:283]
    bt[:, BIAS_COL["bc0"]] = inputs["bc0"][e]
    blob[:, WC1_OFF: WC1_OFF + 3] = inputs["wc1"][e]
    bt[0:3, BIAS_COL["bc1"]] = inputs["bc1"][e]


# ---------------------------------------------------------------------------
# Device program
# ---------------------------------------------------------------------------

def _build_program(C, nslot, group=2, wbufs=4):
    """Build the SPMD Bass program: nslot expert slots of C tokens each."""
    nall = nslot * C
    f32 = mybir.dt.float32
    bf16 = mybir.dt.bfloat16
    Sin = mybir.ActivationFunctionType.Sin
    Sigmoid = mybir.ActivationFunctionType.Sigmoid
    Relu = mybir.ActivationFunctionType.Relu
    Identity = mybir.ActivationFunctionType.Identity
    ADD = mybir.AluOpType.add
    SUB = mybir.AluOpType.subtract
    MUL = mybir.AluOpType.mult
    MAX = mybir.AluOpType.max
    MIN = mybir.AluOpType.min
    # range-reduction constants (Cody-Waite, fp32 magic rounding)
    INV2PI = float(np.float32(1.0 / (2 * PI)))
    MAGIC = 12582912.0            # 1.5 * 2**23: forces round-to-int in fp32
    C1 = 6.28125                  # 2*pi high part, exact in fp32
    C2 = float(np.float32(2 * PI - 6.28125))
    CLAMP = 3.1415925             # just under pi (ACT Sin domain is [-pi, pi])
    HALF_PI = float(np.float32(PI / 2))

    nc = bacc.Bacc("TRN2", target_bir_lowering=False, debug=False)
    wt_d = nc.dram_tensor("wt", (nslot, 128, NCOLS), bf16, kind="ExternalInput")
    bt_d = nc.dram_tensor("bt", (128, nslot * NB), f32, kind="ExternalInput")
    pts_d = nc.dram_tensor("ptsT", (3, nall), f32, kind="ExternalInput")
    dir_d = nc.dram_tensor("dirT", (3, nall), f32, kind="ExternalInput")
    fx_d = nc.dram_tensor("fx", (3, 18), f32, kind="ExternalInput")
    fd_d = nc.dram_tensor("fd", (3, 12), f32, kind="ExternalInput")
    al_d = nc.dram_tensor("alpha_out", (1, nall), f32, kind="ExternalOutput")
    co_d = nc.dram_tensor("color_out", (3, nall), f32, kind="ExternalOutput")

    # DVE vs ACT load balancing for the psum->sbuf bias+activation moves
    eng_t = {"dve": 0.0, "act": 0.0}

    def pick_engine(n_free):
        dve_cost = n_free / 0.96 + 90.0
        act_cost = (n_free + 352) / 1.2
        if eng_t["dve"] + dve_cost <= eng_t["act"] + act_cost:
            eng_t["dve"] += dve_cost
            return "dve"
        eng_t["act"] += act_cost
        return "act"

    with tile.TileContext(nc) as tc:
        with (
            tc.tile_pool(name="wp", bufs=wbufs) as wp,
            tc.tile_pool(name="cp", bufs=1) as cp,
            tc.tile_pool(name="xp", bufs=3 * group + 2) as xp,
            tc.tile_pool(name="psA", bufs=6, space=bass.MemorySpace.PSUM) as psA,
            tc.tile_pool(name="psB", bufs=2, space=bass.MemorySpace.PSUM) as psB,
        ):
            # ---- constants + embeddings (once per core) ----
            embP = cp.tile([67, nall], bf16)  # points: sin 0:18, cos 32:50, xyz 64:67
            embD = cp.tile([67, nall], bf16)  # dirs:   sin 0:12, cos 32:44, xyz 64:67
            nc.vector.memset(embP[:], 0.0)
            nc.vector.memset(embD[:], 0.0)
            fx_sb = cp.tile([3, 18], f32)
            nc.sync.dma_start(fx_sb[:], fx_d.ap()[:])
            fd_sb = cp.tile([3, 12], f32)
            nc.sync.dma_start(fd_sb[:], fd_d.ap()[:])
            pts_sb = cp.tile([3, nall], f32)
            nc.sync.dma_start(pts_sb[:], pts_d.ap()[:])
            dir_sb = cp.tile([3, nall], f32)
            nc.sync.dma_start(dir_sb[:], dir_d.ap()[:])
            bt_sb = cp.tile([128, nslot * NB], f32)
            nc.sync.dma_start(bt_sb[:], bt_d.ap()[:])

            nc.vector.tensor_copy(embP[64:67, :], pts_sb[:])
            nc.vector.tensor_copy(embD[64:67, :], dir_sb[:])

            alpha_sb = cp.tile([1, nall], f32)
            color_sb = cp.tile([3, nall], f32)

            # frequency expansion + range-reduced sin/cos, in <=512-col chunks.
            # phase t lives in psum; v = t - round(t/2pi)*2pi via magic-constant
            # rounding + two-step Cody-Waite, clamped into ACT Sin's domain.
            def reduce_sin(tsrc, rows, ncol):
                t1 = xp.tile([rows, ncol], f32, tag="vred")
                nc.vector.tensor_scalar(t1[:], tsrc, INV2PI, MAGIC, MUL, ADD)
                r = xp.tile([rows, ncol], f32, tag="vred")
                nc.vector.tensor_scalar(r[:], t1[:], MAGIC, None, SUB)
                rd = xp.tile([rows, ncol], f32, tag="vred")
                nc.vector.scalar_tensor_tensor(rd[:], r[:], -C1, tsrc, MUL, ADD)
                rd2 = xp.tile([rows, ncol], f32, tag="vred")
                nc.vector.scalar_tensor_tensor(rd2[:], r[:], -C2, rd[:], MUL, ADD)
                v = xp.tile([rows, ncol], f32, tag="vred")
                nc.vector.tensor_scalar(v[:], rd2[:], CLAMP, -CLAMP, MIN, MAX)
                return v

            for lo in range(0, nall, 512):
                hi = min(nall, lo + 512)
                w_ = hi - lo
                for (rows, fmat, src, dst) in (
                    (18, fx_sb, pts_sb, embP),
                    (12, fd_sb, dir_sb, embD),
                ):
                    ep = psA.tile([rows, w_], f32, tag="mlp")
                    nc.tensor.matmul(ep[:], fmat[:, 0:rows], src[:, lo:hi],
                                     start=True, stop=True)
                    vs = reduce_sin(ep[:], rows, w_)
                    nc.scalar.activation(dst[0:rows, lo:hi], vs[:], Sin)
                    pre = xp.tile([rows, w_], f32, tag="vred")
                    nc.vector.tensor_scalar(pre[:], ep[:], HALF_PI, None, ADD)
                    vc = reduce_sin(pre[:], rows, w_)
                    cs = xp.tile([rows, w_], f32, tag="vred")
                    nc.scalar.activation(cs[:], vc[:], Sin)
                    nc.vector.tensor_copy(dst[32:32 + rows, lo:hi], cs[:])

            # ---- per-expert-slot MLP, experts interleaved in groups ----
            def bias_ap(s, col, p=128):
                return bt_sb[0:p, s * NB + col: s * NB + col + 1]

            def move(out_ap, in_ap, b_ap, relu, n_free):
                """psum -> sbuf with fp32 bias add, optional relu; DVE or ACT."""
                if pick_engine(n_free) == "dve":
                    if relu:
                        nc.vector.tensor_scalar(out_ap, in_ap, b_ap, 0.0, ADD, MAX)
                    else:
                        nc.vector.tensor_scalar(out_ap, in_ap, b_ap, None, ADD)
                else:
                    nc.scalar.activation(out_ap, in_ap, Relu if relu else Identity,
                                         bias=b_ap)

            def stage_mid(w, xin, off):
                ps = psA.tile([128, 2 * C], f32, tag="mlp")
                for j in (0, 1):
                    nc.tensor.matmul(ps[:, j * C:(j + 1) * C],
                                     w[0:128, off + j * 128: off + j * 128 + 128],
                                     xin[:, 0, :], start=True, stop=False)
                    nc.tensor.matmul(ps[:, j * C:(j + 1) * C],
                                     w[0:128, off + 256 + j * 128: off + 256 + j * 128 + 128],
                                     xin[:, 1, :], start=False, stop=True)
                return ps

            def relu_out(s, ps, bcol, relu=True):
                xn = xp.tile([128, 2, C], bf16, tag="x")
                for j in (0, 1):
                    move(xn[:, j, :], ps[:, j * C:(j + 1) * C],
                         bias_ap(s, bcol + j), relu, C)
                return xn

            class SlotState:
                pass

            states = [SlotState() for _ in range(nslot)]

            def emit_stage(s, st, stage):
                w = st.w
                sl = slice(s * C, (s + 1) * C)
                if stage == 0:  # L0
                    ps = psA.tile([128, 2 * C], f32, tag="mlp")
                    for j in (0, 1):
                        nc.tensor.matmul(ps[:, j * C:(j + 1) * C],
                                         w[0:67, j * 128: j * 128 + 128],
                                         embP[0:67, sl], start=True, stop=True)
                    st.x = relu_out(s, ps, BIAS_COL[0])
                elif stage in (1, 2, 3, 4, 6, 7):  # mid layers
                    ps = stage_mid(w, st.x, MID_OFF[stage])
                    st.x = relu_out(s, ps, BIAS_COL[stage])
                elif stage == 5:  # skip layer
                    ps = psA.tile([128, 2 * C], f32, tag="mlp")
                    for j in (0, 1):
                        pj = ps[:, j * C:(j + 1) * C]
                        nc.tensor.matmul(pj, w[0:128, L5_OFF + j * 128: L5_OFF + j * 128 + 128],
                                         st.x[:, 0, :], start=True, stop=False)
                        nc.tensor.matmul(pj, w[0:128, L5_OFF + 256 + j * 128: L5_OFF + 256 + j * 128 + 128],
                                         st.x[:, 1, :], start=False, stop=False)
                        nc.tensor.matmul(pj, w[0:67, L5_OFF + 512 + j * 128: L5_OFF + 512 + j * 128 + 128],
                                         embP[0:67, sl], start=False, stop=True)
                    st.x = relu_out(s, ps, BIAS_COL[5])
                elif stage == 8:  # wi -> inter (bias, no relu)
                    ps = stage_mid(w, st.x, WI_OFF)
                    st.it = relu_out(s, ps, BIAS_COL["wi"], relu=False)
                elif stage == 9:  # wa -> alpha
                    pa = psB.tile([3, C], f32, tag="head")
                    nc.tensor.matmul(pa[0:1, :], w[0:128, WA_OFF: WA_OFF + 1],
                                     st.x[:, 0, :], start=True, stop=False)
                    nc.tensor.matmul(pa[0:1, :], w[0:128, WA_OFF + 1: WA_OFF + 2],
                                     st.x[:, 1, :], start=False, stop=True)
                    nc.vector.tensor_scalar(alpha_sb[0:1, sl], pa[0:1, :],
                                            bias_ap(s, BIAS_COL["ba"], p=1), None, ADD)
                elif stage == 10:  # wc0 -> c (relu)
                    pc = psA.tile([128, C], f32, tag="mlp")
                    nc.tensor.matmul(pc[:], w[0:128, WC0_OFF: WC0_OFF + 128],
                                     st.it[:, 0, :], start=True, stop=False)
                    nc.tensor.matmul(pc[:], w[0:128, WC0_OFF + 128: WC0_OFF + 256],
                                     st.it[:, 1, :], start=False, stop=False)
                    nc.tensor.matmul(pc[:], w[0:67, WC0_OFF + 256: WC0_OFF + 384],
                                     embD[0:67, sl], start=False, stop=True)
                    ct = xp.tile([128, C], bf16, tag="ct")
                    move(ct[:], pc[:], bias_ap(s, BIAS_COL["bc0"]), True, C)
                    st.ct = ct
                elif stage == 11:  # wc1 -> sigmoid color
                    pcol = psB.tile([3, C], f32, tag="head")
                    nc.tensor.matmul(pcol[:], w[0:128, WC1_OFF: WC1_OFF + 3],
                                     st.ct[:], start=True, stop=True)
                    nc.scalar.activation(color_sb[0:3, sl], pcol[:], Sigmoid,
                                         bias=bias_ap(s, BIAS_COL["bc1"], p=3))

            for g0 in range(0, nslot, group):
                slots = range(g0, min(g0 + group, nslot))
                for s in slots:
                    st = states[s]
                    st.w = wp.tile([128, NCOLS], bf16, tag="w")
                    nc.sync.dma_start(st.w[:], wt_d.ap()[s])
                for stage in range(12):
                    for s in slots:
                        emit_stage(s, states[s], stage)

            nc.sync.dma_start(al_d.ap()[:], alpha_sb[:])
            nc.sync.dma_start(co_d.ap()[:], color_sb[:])

    nc.compile()
    return nc


_prog_cache = {}
_last_results = None


def _get_program(C, nslot):
    key = (C, nslot)
    if key not in _prog_cache:
        _prog_cache[key] = _build_program(C, nslot)
    return _prog_cache[key]


# ---------------------------------------------------------------------------
# Host wrapper
# ---------------------------------------------------------------------------

def kernel(**inputs):
    global _last_results
    inputs = {k: np.asarray(v) for k, v in inputs.items()}
    idx = inputs["index"].astype(np.int64)
    B = idx.shape[0]
    points = inputs["points"].astype(np.float32)
    dirs = inputs["directions"].astype(np.float32)

    # --- routing: split each expert's tokens into <=CAP_MAX chunks ("virtual
    # experts"), distribute round-robin (sorted by size) over 8 cores ---
    tok = [np.nonzero(idx == e)[0] for e in range(E)]
    virt = []  # (expert, token_ids)
    for e in range(E):
        t = tok[e]
        if len(t) == 0:
            continue
        for lo in range(0, len(t), CAP_MAX):
            virt.append((e, t[lo: lo + CAP_MAX]))
    if not virt:
        virt = [(0, np.zeros((0,), np.int64))]
    virt.sort(key=lambda v: -len(v[1]))
    nslot = max(1, int(np.ceil(len(virt) / N_CORES)))
    C = max(4, int(np.ceil(max(len(v[1]) for v in virt) / 4) * 4))
    nall = nslot * C

    core_slots = [[] for _ in range(N_CORES)]
    for i, v in enumerate(virt):
        core_slots[i % N_CORES].append(v)

    nc = _get_program(C, nslot)

    fx = np.zeros((3, 18), np.float32)
    for c in range(3):
        for k in range(NX):
            fx[c, c * NX + k] = float(2 ** k)
    fd = np.zeros((3, 12), np.float32)
    for c in range(3):
        for k in range(ND):
            fd[c, c * ND + k] = float(2 ** k)

    in_maps = []
    for c in range(N_CORES):
        wt = np.zeros((nslot, 128, NCOLS), ml_dtypes.bfloat16)
        bt = np.zeros((128, nslot * NB), np.float32)
        ptsT = np.zeros((3, nall), np.float32)
        dirT = np.zeros((3, nall), np.float32)
        for s, (e, t) in enumerate(core_slots[c]):
            _pack_expert(wt[s], bt[:, s * NB:(s + 1) * NB], inputs, e)
            n = len(t)
            if n:
                ptsT[:, s * C: s * C + n] = points[t].T
                dirT[:, s * C: s * C + n] = dirs[t].T
        in_maps.append({"wt": wt, "bt": bt, "ptsT": ptsT, "dirT": dirT,
                        "fx": fx, "fd": fd})

    res = run_bass_kernel_spmd(nc, in_maps, core_ids=list(range(N_CORES)))
    _last_results = res

    out = np.zeros((B, 4), np.float32)
    for c in range(N_CORES):
        al = res.results[c]["alpha_out"]
        co = res.results[c]["color_out"]
        for s, (e, t) in enumerate(core_slots[c]):
            n = len(t)
            if n:
                out[t, 0] = al[0, s * C: s * C + n]
                out[t, 1:4] = co[:, s * C: s * C + n].T
    return out


# revision 9
# speedup vs baseline: 3.1238x; 1.0760x over previous
"""NerfExperts MoE kernel for Trainium2, expert-parallel over 8 NeuronCores.

Strategy: each of the 1024 points is routed to one of 100 experts
(~2.3MB of fp32 weights each, ~232MB total -> memory bound).  We shard
the *experts* across the 8 cores (13 slots per core), dispatch tokens to
their expert's core on the host, and stream each expert's weights from
HBM exactly once, as bf16 (halves traffic, halves PE weight-load time).
Activations stay transposed ([feature, token]) so every layer is a plain
stationary-weight matmul with no transposes.  Experts advance through
the MLP in lockstep "waves" that share PSUM tiles, so the PSUM->SBUF
bias+activation moves are batched across a whole wave (per-expert biases
applied via stride-0 broadcast APs on DVE).  Matmul accumulation is fp32
in PSUM; biases are applied in fp32.  Harmonic-embedding phases are
computed in fp32 with Cody-Waite range reduction for ACT's Sin.
"""

import numpy as np
import ml_dtypes

import concourse.bass as bass
import concourse.bacc as bacc
import concourse.mybir as mybir
import concourse.tile as tile
from concourse.bass_utils import run_bass_kernel_spmd

PI = float(np.pi)
N_CORES = 8
E = 100
NX, ND = 6, 4
CAP_MAX = 128  # max tokens per expert slot (keeps matmul N and PSUM in range)

# ---------------------------------------------------------------------------
# Weight blob column layout (bf16 columns within a [128, NCOLS] per-expert
# blob).  Row convention for embedding-consuming slabs: sin rows at
# partitions 0:18 (0:12 for dirs), cos at 32:50 (32:44), xyz at 64:67,
# dead rows zeroed.  Biases live in a separate fp32 tensor.
# ---------------------------------------------------------------------------
L0_W = 0                                 # [0:256) w0 slab
_off = 256
MID_OFF = {}
for _l in (1, 2, 3, 4):
    MID_OFF[_l] = _off
    _off += 512                          # 2 k-slabs x 256
L5_OFF = _off
_off += 768                              # A(256) B(256) skip(256)
for _l in (6, 7):
    MID_OFF[_l] = _off
    _off += 512
WI_OFF = _off
_off += 512
WA_OFF = _off
_off += 2                                # 2 weight cols
WC0_OFF = _off
_off += 384                              # A(128) B(128) rays(128)
WC1_OFF = _off
_off += 3                                # 3 weight cols
NCOLS = _off                             # 4997

# fp32 bias tensor [128, 21*nslot], layer-major columns:
#   mlp stage lidx in 0..8 (layers 0-7, then wi): col = lidx*2*nslot + s*2 + j
#   ba: 18*nslot + s ; bc0: 19*nslot + s ; bc1: 20*nslot + s
NB = 21


def _pack_expert(blob, bt, s, nslot, inputs, e):
    """Fill one [128, NCOLS] bf16 blob + slot s columns of bt [128, 21*nslot]."""
    n2 = 2 * nslot

    def set_b2(lidx, b):
        bt[:, lidx * n2 + s * 2] = b[0:128]
        bt[:, lidx * n2 + s * 2 + 1] = b[128:256]

    w0 = inputs["w0"][e]                             # [39, 256]
    blob[0:18, 0:256] = w0[0:18]
    blob[32:50, 0:256] = w0[18:36]
    blob[64:67, 0:256] = w0[36:39]
    set_b2(0, inputs["b0"][e])
    for l in (1, 2, 3, 4, 6, 7):
        w = inputs[f"w{l}"][e]                       # [256, 256]
        o = MID_OFF[l]
        for k in (0, 1):
            blob[:, o + k * 256: o + (k + 1) * 256] = w[128 * k: 128 * (k + 1)]
        set_b2(l, inputs[f"b{l}"][e])
    w5 = inputs["w5"][e]                             # [295, 256]
    for k in (0, 1):
        blob[:, L5_OFF + k * 256: L5_OFF + (k + 1) * 256] = w5[128 * k: 128 * (k + 1)]
    blob[0:18, L5_OFF + 512: L5_OFF + 768] = w5[256:274]
    blob[32:50, L5_OFF + 512: L5_OFF + 768] = w5[274:292]
    blob[64:67, L5_OFF + 512: L5_OFF + 768] = w5[292:295]
    set_b2(5, inputs["b5"][e])
    wi = inputs["wi"][e]
    for k in (0, 1):
        blob[:, WI_OFF + k * 256: WI_OFF + (k + 1) * 256] = wi[128 * k: 128 * (k + 1)]
    set_b2(8, inputs["bi"][e])
    wa = inputs["wa"][e][:, 0]                       # [256]
    blob[:, WA_OFF] = wa[0:128]
    blob[:, WA_OFF + 1] = wa[128:256]
    bt[0, 18 * nslot + s] = inputs["ba"][e][0]
    wc0 = inputs["wc0"][e]                           # [283, 128]
    blob[:, WC0_OFF: WC0_OFF + 128] = wc0[0:128]
    blob[:, WC0_OFF + 128: WC0_OFF + 256] = wc0[128:256]
    blob[0:12, WC0_OFF + 256: WC0_OFF + 384] = wc0[256:268]
    blob[32:44, WC0_OFF + 256: WC0_OFF + 384] = wc0[268:280]
    blob[64:67, WC0_OFF + 256: WC0_OFF + 384] = wc0[280:283]
    bt[:, 19 * nslot + s] = inputs["bc0"][e]
    blob[:, WC1_OFF: WC1_OFF + 3] = inputs["wc1"][e]
    bt[0:3, 20 * nslot + s] = inputs["bc1"][e]


def _make_waves(nslot, C):
    gmax = max(1, min(512 // (2 * C), 6))
    nw = int(np.ceil(nslot / gmax))
    base = nslot // nw
    rem = nslot - base * nw
    sizes = [base + (1 if i < rem else 0) for i in range(nw)]
    waves, s0 = [], 0
    for g in sizes:
        waves.append((s0, s0 + g))
        s0 += g
    return waves


# ---------------------------------------------------------------------------
# Device program
# ---------------------------------------------------------------------------

def _build_program(C, nslot):
    """Build the SPMD Bass program: nslot expert slots of C tokens each."""
    nall = nslot * C
    waves = _make_waves(nslot, C)
    nw = len(waves)
    f32 = mybir.dt.float32
    bf16 = mybir.dt.bfloat16
    Sin = mybir.ActivationFunctionType.Sin
    Sigmoid = mybir.ActivationFunctionType.Sigmoid
    Relu = mybir.ActivationFunctionType.Relu
    ADD = mybir.AluOpType.add
    SUB = mybir.AluOpType.subtract
    MUL = mybir.AluOpType.mult
    MAX = mybir.AluOpType.max
    MIN = mybir.AluOpType.min
    # range-reduction constants (Cody-Waite, fp32 magic rounding)
    INV2PI = float(np.float32(1.0 / (2 * PI)))
    MAGIC = 12582912.0            # 1.5 * 2**23: forces round-to-int in fp32
    C1 = 6.28125                  # 2*pi high part, exact in fp32
    C2 = float(np.float32(2 * PI - 6.28125))
    CLAMP = 3.1415925             # just under pi (ACT Sin domain is [-pi, pi])
    HALF_PI = float(np.float32(PI / 2))

    nc = bacc.Bacc("TRN2", target_bir_lowering=False, debug=False)
    wt_d = nc.dram_tensor("wt", (128, nslot * NCOLS), bf16, kind="ExternalInput")
    bt_d = nc.dram_tensor("bt", (128, NB * nslot), f32, kind="ExternalInput")
    pts_d = nc.dram_tensor("ptsT", (3, nall), f32, kind="ExternalInput")
    dir_d = nc.dram_tensor("dirT", (3, nall), f32, kind="ExternalInput")
    fx_d = nc.dram_tensor("fx", (3, 18), f32, kind="ExternalInput")
    fd_d = nc.dram_tensor("fd", (3, 12), f32, kind="ExternalInput")
    al_d = nc.dram_tensor("alpha_out", (1, nall), f32, kind="ExternalOutput")
    co_d = nc.dram_tensor("color_out", (3, nall), f32, kind="ExternalOutput")

    with tile.TileContext(nc) as tc:
        with (
            tc.tile_pool(name="wp", bufs=nw) as wp,
            tc.tile_pool(name="cp", bufs=1) as cp,
            tc.tile_pool(name="xp", bufs=2 * nw + 2) as xp,
            tc.tile_pool(name="psA", bufs=6, space=bass.MemorySpace.PSUM) as psA,
            tc.tile_pool(name="psB", bufs=2, space=bass.MemorySpace.PSUM) as psB,
        ):
            # ---- wave weight DMAs (one big transfer per wave) ----
            gw_max = max(s1 - s0 for s0, s1 in waves)
            wtiles = []
            for (s0, s1) in waves:
                wv = wp.tile([128, (s1 - s0) * NCOLS], bf16, tag="w")
                nc.sync.dma_start(wv[:], wt_d.ap()[:, s0 * NCOLS: s1 * NCOLS])
                wtiles.append(wv)

            # ---- constants + embeddings (once per core) ----
            embP = cp.tile([67, nall], bf16)  # points: sin 0:18, cos 32:50, xyz 64:67
            embD = cp.tile([67, nall], bf16)  # dirs:   sin 0:12, cos 32:44, xyz 64:67
            nc.vector.memset(embP[:], 0.0)
            nc.vector.memset(embD[:], 0.0)
            fx_sb = cp.tile([3, 18], f32)
            nc.sync.dma_start(fx_sb[:], fx_d.ap()[:])
            fd_sb = cp.tile([3, 12], f32)
            nc.sync.dma_start(fd_sb[:], fd_d.ap()[:])
            pts_sb = cp.tile([3, nall], f32)
            nc.sync.dma_start(pts_sb[:], pts_d.ap()[:])
            dir_sb = cp.tile([3, nall], f32)
            nc.sync.dma_start(dir_sb[:], dir_d.ap()[:])
            bt_sb = cp.tile([128, NB * nslot], f32)
            nc.sync.dma_start(bt_sb[:], bt_d.ap()[:])

            nc.vector.tensor_copy(embP[64:67, :], pts_sb[:])
            nc.vector.tensor_copy(embD[64:67, :], dir_sb[:])

            alpha_sb = cp.tile([1, nall], f32)
            color_sb = cp.tile([3, nall], f32)

            # frequency expansion + range-reduced sin/cos, in <=512-col chunks.
            def reduce_sin(tsrc, rows, ncol):
                t1 = xp.tile([rows, ncol], f32, tag="vred")
                nc.vector.tensor_scalar(t1[:], tsrc, INV2PI, MAGIC, MUL, ADD)
                r = xp.tile([rows, ncol], f32, tag="vred")
                nc.vector.tensor_scalar(r[:], t1[:], MAGIC, None, SUB)
                rd = xp.tile([rows, ncol], f32, tag="vred")
                nc.vector.scalar_tensor_tensor(rd[:], r[:], -C1, tsrc, MUL, ADD)
                rd2 = xp.tile([rows, ncol], f32, tag="vred")
                nc.vector.scalar_tensor_tensor(rd2[:], r[:], -C2, rd[:], MUL, ADD)
                v = xp.tile([rows, ncol], f32, tag="vred")
                nc.vector.tensor_scalar(v[:], rd2[:], CLAMP, -CLAMP, MIN, MAX)
                return v

            for lo in range(0, nall, 512):
                hi = min(nall, lo + 512)
                w_ = hi - lo
                for (rows, fmat, src, dst) in (
                    (18, fx_sb, pts_sb, embP),
                    (12, fd_sb, dir_sb, embD),
                ):
                    ep = psA.tile([rows, w_], f32, tag="mlp")
                    nc.tensor.matmul(ep[:], fmat[:, 0:rows], src[:, lo:hi],
                                     start=True, stop=True)
                    vs = reduce_sin(ep[:], rows, w_)
                    nc.scalar.activation(dst[0:rows, lo:hi], vs[:], Sin)
                    pre = xp.tile([rows, w_], f32, tag="vred")
                    nc.vector.tensor_scalar(pre[:], ep[:], HALF_PI, None, ADD)
                    vc = reduce_sin(pre[:], rows, w_)
                    cs = xp.tile([rows, w_], f32, tag="vred")
                    nc.scalar.activation(cs[:], vc[:], Sin)
                    nc.vector.tensor_copy(dst[32:32 + rows, lo:hi], cs[:])

            # ---- wave-lockstep MLP ----
            def bias2_bcast(lidx, s0, s1):
                g = s1 - s0
                ap = bt_sb[:, lidx * 2 * nslot + s0 * 2: lidx * 2 * nslot + s1 * 2]
                return ap.rearrange("p (g j) -> p j g", j=2).broadcast_to(
                    [128, 2, g, C])

            def bias1_bcast(which, s0, s1, p=128):
                g = s1 - s0
                ap = bt_sb[0:p, which * nslot + s0: which * nslot + s1]
                return ap.broadcast_to([p, g, C])

            # per-wave state
            xs = [None] * nw      # current activation [128, 2, g*C] bf16
            its = [None] * nw
            cts = [None] * nw

            def mm_mid(wi_, off, ps, xin, g):
                wv = wtiles[wi_]
                for i in range(g):
                    ob = i * NCOLS
                    for j in (0, 1):
                        pj = ps[:, j, i * C:(i + 1) * C]
                        nc.tensor.matmul(pj, wv[0:128, ob + off + j * 128: ob + off + j * 128 + 128],
                                         xin[:, 0, i * C:(i + 1) * C],
                                         start=True, stop=False)
                        nc.tensor.matmul(pj, wv[0:128, ob + off + 256 + j * 128: ob + off + 256 + j * 128 + 128],
                                         xin[:, 1, i * C:(i + 1) * C],
                                         start=False, stop=True)

            def move2(ps, lidx, s0, s1, relu=True):
                """psum [128,2,g*C] -> new bf16 x tile, bias add (+relu)."""
                g = s1 - s0
                xn = xp.tile([128, 2, g * C], bf16, tag="x")
                psv = ps[:].rearrange("p j (g c) -> p j g c", g=g)
                xnv = xn[:].rearrange("p j (g c) -> p j g c", g=g)
                nc.vector.tensor_tensor(xnv, psv, bias2_bcast(lidx, s0, s1), ADD)
                if relu:
                    nc.scalar.activation(xn[:], xn[:], Relu)
                return xn

            def emit_stage(wi_, stage):
                s0, s1 = waves[wi_]
                g = s1 - s0
                wv = wtiles[wi_]
                if stage == 0:  # L0
                    ps = psA.tile([128, 2, g * C], f32, tag="mlp")
                    for i in range(g):
                        ob = i * NCOLS
                        sl = slice((s0 + i) * C, (s0 + i + 1) * C)
                        for j in (0, 1):
                            nc.tensor.matmul(ps[:, j, i * C:(i + 1) * C],
                                             wv[0:67, ob + j * 128: ob + j * 128 + 128],
                                             embP[0:67, sl], start=True, stop=True)
                    xs[wi_] = move2(ps, 0, s0, s1)
                elif stage in (1, 2, 3, 4, 6, 7):
                    ps = psA.tile([128, 2, g * C], f32, tag="mlp")
                    mm_mid(wi_, MID_OFF[stage], ps, xs[wi_], g)
                    xs[wi_] = move2(ps, stage, s0, s1)
                elif stage == 5:
                    ps = psA.tile([128, 2, g * C], f32, tag="mlp")
                    xin = xs[wi_]
                    for i in range(g):
                        ob = i * NCOLS
                        sl = slice((s0 + i) * C, (s0 + i + 1) * C)
                        for j in (0, 1):
                            pj = ps[:, j, i * C:(i + 1) * C]
                            nc.tensor.matmul(pj, wv[0:128, ob + L5_OFF + j * 128: ob + L5_OFF + j * 128 + 128],
                                             xin[:, 0, i * C:(i + 1) * C],
                                             start=True, stop=False)
                            nc.tensor.matmul(pj, wv[0:128, ob + L5_OFF + 256 + j * 128: ob + L5_OFF + 256 + j * 128 + 128],
                                             xin[:, 1, i * C:(i + 1) * C],
                                             start=False, stop=False)
                            nc.tensor.matmul(pj, wv[0:67, ob + L5_OFF + 512 + j * 128: ob + L5_OFF + 512 + j * 128 + 128],
                                             embP[0:67, sl], start=False, stop=True)
                    xs[wi_] = move2(ps, 5, s0, s1)
                elif stage == 8:  # wi -> inter (bias, no relu)
                    ps = psA.tile([128, 2, g * C], f32, tag="mlp")
                    mm_mid(wi_, WI_OFF, ps, xs[wi_], g)
                    its[wi_] = move2(ps, 8, s0, s1, relu=False)
                elif stage == 9:  # wa -> alpha
                    pa = psB.tile([3, g * C], f32, tag="head")
                    xin = xs[wi_]
                    for i in range(g):
                        ob = i * NCOLS
                        nc.tensor.matmul(pa[0:1, i * C:(i + 1) * C],
                                         wv[0:128, ob + WA_OFF: ob + WA_OFF + 1],
                                         xin[:, 0, i * C:(i + 1) * C],
                                         start=True, stop=False)
                        nc.tensor.matmul(pa[0:1, i * C:(i + 1) * C],
                                         wv[0:128, ob + WA_OFF + 1: ob + WA_OFF + 2],
                                         xin[:, 1, i * C:(i + 1) * C],
                                         start=False, stop=True)
                    av = alpha_sb[0:1, s0 * C: s1 * C].rearrange(
                        "p (g c) -> p g c", g=g)
                    pav = pa[0:1, :].rearrange("p (g c) -> p g c", g=g)
                    nc.vector.tensor_tensor(av, pav, bias1_bcast(18, s0, s1, p=1), ADD)
                elif stage == 10:  # wc0 -> c (relu)
                    pc = psA.tile([128, g * C], f32, tag="mlp")
                    it = its[wi_]
                    for i in range(g):
                        ob = i * NCOLS
                        sl = slice((s0 + i) * C, (s0 + i + 1) * C)
                        pj = pc[:, i * C:(i + 1) * C]
                        nc.tensor.matmul(pj, wv[0:128, ob + WC0_OFF: ob + WC0_OFF + 128],
                                         it[:, 0, i * C:(i + 1) * C],
                                         start=True, stop=False)
                        nc.tensor.matmul(pj, wv[0:128, ob + WC0_OFF + 128: ob + WC0_OFF + 256],
                                         it[:, 1, i * C:(i + 1) * C],
                                         start=False, stop=False)
                        nc.tensor.matmul(pj, wv[0:67, ob + WC0_OFF + 256: ob + WC0_OFF + 384],
                                         embD[0:67, sl], start=False, stop=True)
                    ct = xp.tile([128, g * C], bf16, tag="ct")
                    pcv = pc[:].rearrange("p (g c) -> p g c", g=g)
                    ctv = ct[:].rearrange("p (g c) -> p g c", g=g)
                    nc.vector.tensor_tensor(ctv, pcv, bias1_bcast(19, s0, s1), ADD)
                    nc.scalar.activation(ct[:], ct[:], Relu)
                    cts[wi_] = ct
                elif stage == 11:  # wc1 -> sigmoid color
                    pcol = psB.tile([3, g * C], f32, tag="head")
                    ct = cts[wi_]
                    for i in range(g):
                        ob = i * NCOLS
                        nc.tensor.matmul(pcol[:, i * C:(i + 1) * C],
                                         wv[0:128, ob + WC1_OFF: ob + WC1_OFF + 3],
                                         ct[:, i * C:(i + 1) * C],
                                         start=True, stop=True)
                    ctmp = xp.tile([3, g * C], f32, tag="ctmp")
                    pv = pcol[:].rearrange("p (g c) -> p g c", g=g)
                    cv = ctmp[:].rearrange("p (g c) -> p g c", g=g)
                    nc.vector.tensor_tensor(cv, pv, bias1_bcast(20, s0, s1, p=3), ADD)
                    nc.scalar.activation(color_sb[0:3, s0 * C: s1 * C], ctmp[:],
                                         Sigmoid)

            for stage in range(12):
                for wi_ in range(nw):
                    emit_stage(wi_, stage)

            nc.sync.dma_start(al_d.ap()[:], alpha_sb[:])
            nc.sync.dma_start(co_d.ap()[:], color_sb[:])

    nc.compile()
    return nc


_prog_cache = {}
_last_results = None


def _get_program(C, nslot):
    key = (C, nslot)
    if key not in _prog_cache:
        _prog_cache[key] = _build_program(C, nslot)
    return _prog_cache[key]


# ---------------------------------------------------------------------------
# Host wrapper
# ---------------------------------------------------------------------------

def kernel(**inputs):
    global _last_results
    inputs = {k: np.asarray(v) for k, v in inputs.items()}
    idx = inputs["index"].astype(np.int64)
    B = idx.shape[0]
    points = inputs["points"].astype(np.float32)
    dirs = inputs["directions"].astype(np.float32)

    # --- routing: split each expert's tokens into <=CAP_MAX chunks ("virtual
    # experts"), distribute round-robin (sorted by size) over 8 cores ---
    tok = [np.nonzero(idx == e)[0] for e in range(E)]
    virt = []  # (expert, token_ids)
    for e in range(E):
        t = tok[e]
        if len(t) == 0:
            continue
        for lo in range(0, len(t), CAP_MAX):
            virt.append((e, t[lo: lo + CAP_MAX]))
    if not virt:
        virt = [(0, np.zeros((0,), np.int64))]
    virt.sort(key=lambda v: -len(v[1]))
    nslot = max(1, int(np.ceil(len(virt) / N_CORES)))
    C = max(4, int(np.ceil(max(len(v[1]) for v in virt) / 4) * 4))
    nall = nslot * C

    core_slots = [[] for _ in range(N_CORES)]
    for i, v in enumerate(virt):
        core_slots[i % N_CORES].append(v)

    nc = _get_program(C, nslot)

    fx = np.zeros((3, 18), np.float32)
    for c in range(3):
        for k in range(NX):
            fx[c, c * NX + k] = float(2 ** k)
    fd = np.zeros((3, 12), np.float32)
    for c in range(3):
        for k in range(ND):
            fd[c, c * ND + k] = float(2 ** k)

    in_maps = []
    for c in range(N_CORES):
        wt = np.zeros((128, nslot * NCOLS), ml_dtypes.bfloat16)
        bt = np.zeros((128, NB * nslot), np.float32)
        ptsT = np.zeros((3, nall), np.float32)
        dirT = np.zeros((3, nall), np.float32)
        for s, (e, t) in enumerate(core_slots[c]):
            _pack_expert(wt[:, s * NCOLS:(s + 1) * NCOLS], bt, s, nslot, inputs, e)
            n = len(t)
            if n:
                ptsT[:, s * C: s * C + n] = points[t].T
                dirT[:, s * C: s * C + n] = dirs[t].T
        in_maps.append({"wt": wt, "bt": bt, "ptsT": ptsT, "dirT": dirT,
                        "fx": fx, "fd": fd})

    res = run_bass_kernel_spmd(nc, in_maps, core_ids=list(range(N_CORES)))
    _last_results = res

    out = np.zeros((B, 4), np.float32)
    for c in range(N_CORES):
        al = res.results[c]["alpha_out"]
        co = res.results[c]["color_out"]
        for s, (e, t) in enumerate(core_slots[c]):
            n = len(t)
            if n:
                out[t, 0] = al[0, s * C: s * C + n]
                out[t, 1:4] = co[:, s * C: s * C + n].T
    return out


# revision 12
# speedup vs baseline: 3.5414x; 1.1337x over previous
"""NerfExperts MoE kernel for Trainium2, expert-parallel over 8 NeuronCores.

Strategy: each of the 1024 points is routed to one of 100 experts
(~2.3MB of fp32 weights each, ~232MB total -> memory bound).  We shard
the *experts* across the 8 cores (13 slots per core), dispatch tokens to
their expert's core on the host, and stream each expert's weights from
HBM exactly once, as bf16.  Weights are streamed LAYER-MAJOR (one DMA
chunk per layer covering all local experts, alternating between the two
HWDGE rings) so compute for layer l only waits on chunk l and the
DMA/compute pipeline drains with a one-stage tail.  Activations stay
transposed ([feature, token]); experts advance through the MLP in
lockstep "waves" that share PSUM tiles, so PSUM->SBUF bias+activation
moves are batched across a wave (per-expert fp32 biases via stride-0
broadcast APs on DVE, relu on ACT).  Harmonic-embedding phases are
computed in fp32 with Cody-Waite range reduction for ACT's Sin.
"""

import numpy as np
import ml_dtypes

import concourse.bass as bass
import concourse.bacc as bacc
import concourse.mybir as mybir
import concourse.tile as tile
from concourse.bass_utils import run_bass_kernel_spmd

PI = float(np.pi)
N_CORES = 8
E = 100
NX, ND = 6, 4
CAP_MAX = 128  # max tokens per expert slot (keeps matmul N and PSUM in range)

# Per-slot column widths of each weight chunk (chunk key = mlp stage it feeds;
# stage 9 (wa) rides in chunk 8, stage 11 (wc1) rides in chunk 10).
CHUNKS = [0, 1, 2, 3, 4, 5, 6, 7, 8, 10]
CHUNK_COLS = {0: 256, 1: 512, 2: 512, 3: 512, 4: 512, 5: 768,
              6: 512, 7: 512, 8: 514, 10: 387}

# fp32 bias tensor [128, 21*nslot], layer-major columns:
#   mlp stage lidx in 0..8 (layers 0-7, then wi): col = lidx*2*nslot + s*2 + j
#   ba: 18*nslot + s ; bc0: 19*nslot + s ; bc1: 20*nslot + s
NB = 21


def _pack_expert(wt, bt, s, nslot, inputs, e, coff):
    """Fill slot s columns of the per-stage blocks of wt [128, TOT] (fp32 view)
    and bias columns of bt [128, 21*nslot]."""
    n2 = 2 * nslot

    def blk(st):
        o = coff[st] + s * CHUNK_COLS[st]
        return o

    def set_b2(lidx, b):
        bt[:, lidx * n2 + s * 2] = b[0:128]
        bt[:, lidx * n2 + s * 2 + 1] = b[128:256]

    o = blk(0)
    w0 = inputs["w0"][e]                             # [39, 256]
    wt[0:18, o: o + 256] = w0[0:18]
    wt[32:50, o: o + 256] = w0[18:36]
    wt[64:67, o: o + 256] = w0[36:39]
    set_b2(0, inputs["b0"][e])
    for l in (1, 2, 3, 4, 6, 7):
        w = inputs[f"w{l}"][e]                       # [256, 256]
        o = blk(l)
        for k in (0, 1):
            wt[:, o + k * 256: o + (k + 1) * 256] = w[128 * k: 128 * (k + 1)]
        set_b2(l, inputs[f"b{l}"][e])
    w5 = inputs["w5"][e]                             # [295, 256]
    o = blk(5)
    for k in (0, 1):
        wt[:, o + k * 256: o + (k + 1) * 256] = w5[128 * k: 128 * (k + 1)]
    wt[0:18, o + 512: o + 768] = w5[256:274]
    wt[32:50, o + 512: o + 768] = w5[274:292]
    wt[64:67, o + 512: o + 768] = w5[292:295]
    set_b2(5, inputs["b5"][e])
    o = blk(8)
    wi = inputs["wi"][e]
    for k in (0, 1):
        wt[:, o + k * 256: o + (k + 1) * 256] = wi[128 * k: 128 * (k + 1)]
    set_b2(8, inputs["bi"][e])
    wa = inputs["wa"][e][:, 0]                       # [256]
    wt[:, o + 512] = wa[0:128]
    wt[:, o + 513] = wa[128:256]
    bt[0, 18 * nslot + s] = inputs["ba"][e][0]
    o = blk(10)
    wc0 = inputs["wc0"][e]                           # [283, 128]
    wt[:, o: o + 128] = wc0[0:128]
    wt[:, o + 128: o + 256] = wc0[128:256]
    wt[0:12, o + 256: o + 384] = wc0[256:268]
    wt[32:44, o + 256: o + 384] = wc0[268:280]
    wt[64:67, o + 256: o + 384] = wc0[280:283]
    bt[:, 19 * nslot + s] = inputs["bc0"][e]
    wt[:, o + 384: o + 387] = inputs["wc1"][e]
    bt[0:3, 20 * nslot + s] = inputs["bc1"][e]


def _chunk_offsets(nslot):
    coff, tot = {}, 0
    for st in CHUNKS:
        coff[st] = tot
        tot += nslot * CHUNK_COLS[st]
    return coff, tot


def _make_waves(nslot, C):
    gmax = max(1, min(512 // (2 * C), 6))
    nw = int(np.ceil(nslot / gmax))
    base = nslot // nw
    rem = nslot - base * nw
    sizes = [base + (1 if i < rem else 0) for i in range(nw)]
    waves, s0 = [], 0
    for g in sizes:
        waves.append((s0, s0 + g))
        s0 += g
    return waves


# ---------------------------------------------------------------------------
# Device program
# ---------------------------------------------------------------------------

def _build_program(C, nslot):
    """Build the SPMD Bass program: nslot expert slots of C tokens each."""
    nall = nslot * C
    waves = _make_waves(nslot, C)
    nw = len(waves)
    coff, totcols = _chunk_offsets(nslot)
    f32 = mybir.dt.float32
    bf16 = mybir.dt.bfloat16
    Sin = mybir.ActivationFunctionType.Sin
    Sigmoid = mybir.ActivationFunctionType.Sigmoid
    Relu = mybir.ActivationFunctionType.Relu
    ADD = mybir.AluOpType.add
    SUB = mybir.AluOpType.subtract
    MUL = mybir.AluOpType.mult
    MAX = mybir.AluOpType.max
    MIN = mybir.AluOpType.min
    # range-reduction constants (Cody-Waite, fp32 magic rounding)
    INV2PI = float(np.float32(1.0 / (2 * PI)))
    MAGIC = 12582912.0            # 1.5 * 2**23: forces round-to-int in fp32
    C1 = 6.28125                  # 2*pi high part, exact in fp32
    C2 = float(np.float32(2 * PI - 6.28125))
    CLAMP = 3.1415925             # just under pi (ACT Sin domain is [-pi, pi])
    HALF_PI = float(np.float32(PI / 2))

    nc = bacc.Bacc("TRN2", target_bir_lowering=False, debug=False)
    wt_d = nc.dram_tensor("wt", (128, totcols), bf16, kind="ExternalInput")
    bt_d = nc.dram_tensor("bt", (128, NB * nslot), f32, kind="ExternalInput")
    pts_d = nc.dram_tensor("ptsT", (3, nall), f32, kind="ExternalInput")
    dir_d = nc.dram_tensor("dirT", (3, nall), f32, kind="ExternalInput")
    fx_d = nc.dram_tensor("fx", (3, 18), f32, kind="ExternalInput")
    fd_d = nc.dram_tensor("fd", (3, 12), f32, kind="ExternalInput")
    al_d = nc.dram_tensor("alpha_out", (1, nall), f32, kind="ExternalOutput")
    co_d = nc.dram_tensor("color_out", (3, nall), f32, kind="ExternalOutput")

    with tile.TileContext(nc) as tc:
        with (
            tc.tile_pool(name="cp", bufs=1) as cp,
            tc.tile_pool(name="xp", bufs=2 * nw + 2) as xp,
            tc.tile_pool(name="psA", bufs=6, space=bass.MemorySpace.PSUM) as psA,
            tc.tile_pool(name="psB", bufs=2, space=bass.MemorySpace.PSUM) as psB,
        ):
            # ---- layer-major weight chunk DMAs, alternating HWDGE rings ----
            wts = {}
            for i, st in enumerate(CHUNKS):
                wts[st] = cp.tile([128, nslot * CHUNK_COLS[st]], bf16,
                                  name=f"wt{st}", tag=f"wt{st}")
                eng = nc.sync if i % 2 == 0 else nc.scalar
                eng.dma_start(wts[st][:],
                              wt_d.ap()[:, coff[st]: coff[st] + nslot * CHUNK_COLS[st]])

            def slab(st, s, lo, hi, rows=128):
                o = s * CHUNK_COLS[st]
                return wts[st][0:rows, o + lo: o + hi]

            # ---- constants + embeddings (small inputs via SWDGE) ----
            embP = cp.tile([67, nall], bf16)  # points: sin 0:18, cos 32:50, xyz 64:67
            embD = cp.tile([67, nall], bf16)  # dirs:   sin 0:12, cos 32:44, xyz 64:67
            nc.vector.memset(embP[:], 0.0)
            nc.vector.memset(embD[:], 0.0)
            fx_sb = cp.tile([3, 18], f32)
            nc.gpsimd.dma_start(fx_sb[:], fx_d.ap()[:])
            fd_sb = cp.tile([3, 12], f32)
            nc.gpsimd.dma_start(fd_sb[:], fd_d.ap()[:])
            pts_sb = cp.tile([3, nall], f32)
            nc.gpsimd.dma_start(pts_sb[:], pts_d.ap()[:])
            dir_sb = cp.tile([3, nall], f32)
            nc.gpsimd.dma_start(dir_sb[:], dir_d.ap()[:])
            bt_sb = cp.tile([128, NB * nslot], f32)
            nc.gpsimd.dma_start(bt_sb[:], bt_d.ap()[:])

            nc.vector.tensor_copy(embP[64:67, :], pts_sb[:])
            nc.vector.tensor_copy(embD[64:67, :], dir_sb[:])

            alpha_sb = cp.tile([1, nall], f32)
            color_sb = cp.tile([3, nall], f32)

            # frequency expansion + range-reduced sin/cos, in <=512-col chunks.
            def reduce_sin(tsrc, rows, ncol):
                t1 = xp.tile([rows, ncol], f32, tag="vred")
                nc.vector.tensor_scalar(t1[:], tsrc, INV2PI, MAGIC, MUL, ADD)
                r = xp.tile([rows, ncol], f32, tag="vred")
                nc.vector.tensor_scalar(r[:], t1[:], MAGIC, None, SUB)
                rd = xp.tile([rows, ncol], f32, tag="vred")
                nc.vector.scalar_tensor_tensor(rd[:], r[:], -C1, tsrc, MUL, ADD)
                rd2 = xp.tile([rows, ncol], f32, tag="vred")
                nc.vector.scalar_tensor_tensor(rd2[:], r[:], -C2, rd[:], MUL, ADD)
                v = xp.tile([rows, ncol], f32, tag="vred")
                nc.vector.tensor_scalar(v[:], rd2[:], CLAMP, -CLAMP, MIN, MAX)
                return v

            for lo in range(0, nall, 512):
                hi = min(nall, lo + 512)
                w_ = hi - lo
                for (rows, fmat, src, dst) in (
                    (18, fx_sb, pts_sb, embP),
                    (12, fd_sb, dir_sb, embD),
                ):
                    ep = psA.tile([rows, w_], f32, tag="mlp")
                    nc.tensor.matmul(ep[:], fmat[:, 0:rows], src[:, lo:hi],
                                     start=True, stop=True)
                    vs = reduce_sin(ep[:], rows, w_)
                    nc.scalar.activation(dst[0:rows, lo:hi], vs[:], Sin)
                    pre = xp.tile([rows, w_], f32, tag="vred")
                    nc.vector.tensor_scalar(pre[:], ep[:], HALF_PI, None, ADD)
                    vc = reduce_sin(pre[:], rows, w_)
                    cs = xp.tile([rows, w_], f32, tag="vred")
                    nc.scalar.activation(cs[:], vc[:], Sin)
                    nc.vector.tensor_copy(dst[32:32 + rows, lo:hi], cs[:])

            # ---- wave-lockstep MLP ----
            def bias2_bcast(lidx, s0, s1):
                g = s1 - s0
                ap = bt_sb[:, lidx * 2 * nslot + s0 * 2: lidx * 2 * nslot + s1 * 2]
                return ap.rearrange("p (g j) -> p j g", j=2).broadcast_to(
                    [128, 2, g, C])

            def bias1_bcast(which, s0, s1, p=128):
                g = s1 - s0
                ap = bt_sb[0:p, which * nslot + s0: which * nslot + s1]
                return ap.broadcast_to([p, g, C])

            xs = [None] * nw
            its = [None] * nw
            cts = [None] * nw

            def mm_mid(st, ps, xin, s0, s1):
                for i in range(s1 - s0):
                    s = s0 + i
                    for j in (0, 1):
                        pj = ps[:, j, i * C:(i + 1) * C]
                        nc.tensor.matmul(pj, slab(st, s, j * 128, j * 128 + 128),
                                         xin[:, 0, i * C:(i + 1) * C],
                                         start=True, stop=False)
                        nc.tensor.matmul(pj, slab(st, s, 256 + j * 128, 256 + j * 128 + 128),
                                         xin[:, 1, i * C:(i + 1) * C],
                                         start=False, stop=True)

            def move2(ps, lidx, s0, s1, relu=True):
                g = s1 - s0
                xn = xp.tile([128, 2, g * C], bf16, tag="x")
                psv = ps[:].rearrange("p j (g c) -> p j g c", g=g)
                xnv = xn[:].rearrange("p j (g c) -> p j g c", g=g)
                nc.vector.tensor_tensor(xnv, psv, bias2_bcast(lidx, s0, s1), ADD)
                if relu:
                    nc.scalar.activation(xn[:], xn[:], Relu)
                return xn

            def emit_stage(wi_, stage):
                s0, s1 = waves[wi_]
                g = s1 - s0
                if stage == 0:  # L0
                    ps = psA.tile([128, 2, g * C], f32, tag="mlp")
                    for i in range(g):
                        s = s0 + i
                        sl = slice(s * C, (s + 1) * C)
                        for j in (0, 1):
                            nc.tensor.matmul(ps[:, j, i * C:(i + 1) * C],
                                             slab(0, s, j * 128, j * 128 + 128, rows=67),
                                             embP[0:67, sl], start=True, stop=True)
                    xs[wi_] = move2(ps, 0, s0, s1)
                elif stage in (1, 2, 3, 4, 6, 7):
                    ps = psA.tile([128, 2, g * C], f32, tag="mlp")
                    mm_mid(stage, ps, xs[wi_], s0, s1)
                    xs[wi_] = move2(ps, stage, s0, s1)
                elif stage == 5:
                    ps = psA.tile([128, 2, g * C], f32, tag="mlp")
                    xin = xs[wi_]
                    for i in range(g):
                        s = s0 + i
                        sl = slice(s * C, (s + 1) * C)
                        for j in (0, 1):
                            pj = ps[:, j, i * C:(i + 1) * C]
                            nc.tensor.matmul(pj, slab(5, s, j * 128, j * 128 + 128),
                                             xin[:, 0, i * C:(i + 1) * C],
                                             start=True, stop=False)
                            nc.tensor.matmul(pj, slab(5, s, 256 + j * 128, 256 + j * 128 + 128),
                                             xin[:, 1, i * C:(i + 1) * C],
                                             start=False, stop=False)
                            nc.tensor.matmul(pj, slab(5, s, 512 + j * 128, 512 + j * 128 + 128, rows=67),
                                             embP[0:67, sl], start=False, stop=True)
                    xs[wi_] = move2(ps, 5, s0, s1)
                elif stage == 8:  # wi -> inter (bias, no relu)
                    ps = psA.tile([128, 2, g * C], f32, tag="mlp")
                    mm_mid(8, ps, xs[wi_], s0, s1)
                    its[wi_] = move2(ps, 8, s0, s1, relu=False)
                elif stage == 9:  # wa -> alpha (weights ride in chunk 8)
                    pa = psB.tile([3, g * C], f32, tag="head")
                    xin = xs[wi_]
                    for i in range(g):
                        s = s0 + i
                        nc.tensor.matmul(pa[0:1, i * C:(i + 1) * C],
                                         slab(8, s, 512, 513),
                                         xin[:, 0, i * C:(i + 1) * C],
                                         start=True, stop=False)
                        nc.tensor.matmul(pa[0:1, i * C:(i + 1) * C],
                                         slab(8, s, 513, 514),
                                         xin[:, 1, i * C:(i + 1) * C],
                                         start=False, stop=True)
                    av = alpha_sb[0:1, s0 * C: s1 * C].rearrange(
                        "p (g c) -> p g c", g=g)
                    pav = pa[0:1, :].rearrange("p (g c) -> p g c", g=g)
                    nc.vector.tensor_tensor(av, pav, bias1_bcast(18, s0, s1, p=1), ADD)
                elif stage == 10:  # wc0 -> c (relu)
                    pc = psA.tile([128, g * C], f32, tag="mlp")
                    it = its[wi_]
                    for i in range(g):
                        s = s0 + i
                        sl = slice(s * C, (s + 1) * C)
                        pj = pc[:, i * C:(i + 1) * C]
                        nc.tensor.matmul(pj, slab(10, s, 0, 128),
                                         it[:, 0, i * C:(i + 1) * C],
                                         start=True, stop=False)
                        nc.tensor.matmul(pj, slab(10, s, 128, 256),
                                         it[:, 1, i * C:(i + 1) * C],
                                         start=False, stop=False)
                        nc.tensor.matmul(pj, slab(10, s, 256, 384, rows=67),
                                         embD[0:67, sl], start=False, stop=True)
                    ct = xp.tile([128, g * C], bf16, tag="ct")
                    pcv = pc[:].rearrange("p (g c) -> p g c", g=g)
                    ctv = ct[:].rearrange("p (g c) -> p g c", g=g)
                    nc.vector.tensor_tensor(ctv, pcv, bias1_bcast(19, s0, s1), ADD)
                    nc.scalar.activation(ct[:], ct[:], Relu)
                    cts[wi_] = ct
                elif stage == 11:  # wc1 -> sigmoid color (weights in chunk 10)
                    pcol = psB.tile([3, g * C], f32, tag="head")
                    ct = cts[wi_]
                    for i in range(g):
                        s = s0 + i
                        nc.tensor.matmul(pcol[:, i * C:(i + 1) * C],
                                         slab(10, s, 384, 387),
                                         ct[:, i * C:(i + 1) * C],
                                         start=True, stop=True)
                    ctmp = xp.tile([3, g * C], f32, tag="ctmp")
                    pv = pcol[:].rearrange("p (g c) -> p g c", g=g)
                    cv = ctmp[:].rearrange("p (g c) -> p g c", g=g)
                    nc.vector.tensor_tensor(cv, pv, bias1_bcast(20, s0, s1, p=3), ADD)
                    nc.scalar.activation(color_sb[0:3, s0 * C: s1 * C], ctmp[:],
                                         Sigmoid)

            for stage in range(12):
                for wi_ in range(nw):
                    emit_stage(wi_, stage)

            nc.sync.dma_start(al_d.ap()[:], alpha_sb[:])
            nc.sync.dma_start(co_d.ap()[:], color_sb[:])

    nc.compile()
    return nc


_prog_cache = {}
_last_results = None


def _get_program(C, nslot):
    key = (C, nslot)
    if key not in _prog_cache:
        _prog_cache[key] = _build_program(C, nslot)
    return _prog_cache[key]


# ---------------------------------------------------------------------------
# Host wrapper
# ---------------------------------------------------------------------------

def kernel(**inputs):
    global _last_results
    inputs = {k: np.asarray(v) for k, v in inputs.items()}
    idx = inputs["index"].astype(np.int64)
    B = idx.shape[0]
    points = inputs["points"].astype(np.float32)
    dirs = inputs["directions"].astype(np.float32)

    # --- routing: split each expert's tokens into <=CAP_MAX chunks ("virtual
    # experts"), distribute round-robin (sorted by size) over 8 cores ---
    tok = [np.nonzero(idx == e)[0] for e in range(E)]
    virt = []  # (expert, token_ids)
    for e in range(E):
        t = tok[e]
        if len(t) == 0:
            continue
        for lo in range(0, len(t), CAP_MAX):
            virt.append((e, t[lo: lo + CAP_MAX]))
    if not virt:
        virt = [(0, np.zeros((0,), np.int64))]
    virt.sort(key=lambda v: -len(v[1]))
    nslot = max(1, int(np.ceil(len(virt) / N_CORES)))
    C = max(4, int(np.ceil(max(len(v[1]) for v in virt) / 4) * 4))
    nall = nslot * C

    core_slots = [[] for _ in range(N_CORES)]
    for i, v in enumerate(virt):
        core_slots[i % N_CORES].append(v)

    nc = _get_program(C, nslot)
    coff, totcols = _chunk_offsets(nslot)

    fx = np.zeros((3, 18), np.float32)
    for c in range(3):
        for k in range(NX):
            fx[c, c * NX + k] = float(2 ** k)
    fd = np.zeros((3, 12), np.float32)
    for c in range(3):
        for k in range(ND):
            fd[c, c * ND + k] = float(2 ** k)

    in_maps = []
    for c in range(N_CORES):
        wt = np.zeros((128, totcols), np.float32)
        bt = np.zeros((128, NB * nslot), np.float32)
        ptsT = np.zeros((3, nall), np.float32)
        dirT = np.zeros((3, nall), np.float32)
        for s, (e, t) in enumerate(core_slots[c]):
            _pack_expert(wt, bt, s, nslot, inputs, e, coff)
            n = len(t)
            if n:
                ptsT[:, s * C: s * C + n] = points[t].T
                dirT[:, s * C: s * C + n] = dirs[t].T
        in_maps.append({"wt": wt.astype(ml_dtypes.bfloat16), "bt": bt,
                        "ptsT": ptsT, "dirT": dirT, "fx": fx, "fd": fd})

    res = run_bass_kernel_spmd(nc, in_maps, core_ids=list(range(N_CORES)))
    _last_results = res

    out = np.zeros((B, 4), np.float32)
    for c in range(N_CORES):
        al = res.results[c]["alpha_out"]
        co = res.results[c]["color_out"]
        for s, (e, t) in enumerate(core_slots[c]):
            n = len(t)
            if n:
                out[t, 0] = al[0, s * C: s * C + n]
                out[t, 1:4] = co[:, s * C: s * C + n].T
    return out


# revision 13
# speedup vs baseline: 4.0148x; 1.1337x over previous
"""NerfExperts MoE kernel for Trainium2, expert-parallel over 8 NeuronCores.

Strategy: each of the 1024 points is routed to one of 100 experts
(~2.3MB of fp32 weights each, ~232MB total -> memory bound).  We shard
the *experts* across the 8 cores (13 slots per core), dispatch tokens to
their expert's core on the host, and stream each expert's weights from
HBM exactly once, as bf16.  Weights are streamed LAYER-MAJOR (one DMA
chunk per layer covering all local experts, alternating between the two
HWDGE rings) so compute for layer l only waits on chunk l and the
DMA/compute pipeline drains with a one-stage tail.  Activations stay
transposed ([feature, token]); experts advance through the MLP in
lockstep "waves" that share PSUM tiles, so PSUM->SBUF bias+activation
moves are batched across a wave (per-expert fp32 biases via stride-0
broadcast APs on DVE, relu on ACT).  Harmonic-embedding phases are
computed in fp32 with Cody-Waite range reduction for ACT's Sin.
"""

import numpy as np
import ml_dtypes

import concourse.bass as bass
import concourse.bacc as bacc
import concourse.mybir as mybir
import concourse.tile as tile
from concourse.bass_utils import run_bass_kernel_spmd

PI = float(np.pi)
N_CORES = 8
E = 100
NX, ND = 6, 4
CAP_MAX = 128  # max tokens per expert slot (keeps matmul N and PSUM in range)

# Per-slot column widths of each weight chunk (chunk key = mlp stage it feeds;
# stage 9 (wa) rides in chunk 8, stage 11 (wc1) rides in chunk 10).
CHUNKS = [0, 1, 2, 3, 4, 5, 6, 7, 8, 10]
CHUNK_COLS = {0: 256, 1: 512, 2: 512, 3: 512, 4: 512, 5: 768,
              6: 512, 7: 512, 8: 514, 10: 387}

# fp32 bias tensor [128, 21*nslot], layer-major columns:
#   mlp stage lidx in 0..8 (layers 0-7, then wi): col = lidx*2*nslot + s*2 + j
#   ba: 18*nslot + s ; bc0: 19*nslot + s ; bc1: 20*nslot + s
NB = 21


def _pack_expert(wt, bt, s, nslot, inputs, e, coff):
    """Fill slot s columns of the per-stage blocks of wt [128, TOT] (fp32 view)
    and bias columns of bt [128, 21*nslot]."""
    n2 = 2 * nslot

    def blk(st):
        o = coff[st] + s * CHUNK_COLS[st]
        return o

    def set_b2(lidx, b):
        bt[:, lidx * n2 + s * 2] = b[0:128]
        bt[:, lidx * n2 + s * 2 + 1] = b[128:256]

    o = blk(0)
    w0 = inputs["w0"][e]                             # [39, 256]
    wt[0:18, o: o + 256] = w0[0:18]
    wt[32:50, o: o + 256] = w0[18:36]
    wt[64:67, o: o + 256] = w0[36:39]
    set_b2(0, inputs["b0"][e])
    for l in (1, 2, 3, 4, 6, 7):
        w = inputs[f"w{l}"][e]                       # [256, 256]
        o = blk(l)
        for k in (0, 1):
            wt[:, o + k * 256: o + (k + 1) * 256] = w[128 * k: 128 * (k + 1)]
        set_b2(l, inputs[f"b{l}"][e])
    w5 = inputs["w5"][e]                             # [295, 256]
    o = blk(5)
    for k in (0, 1):
        wt[:, o + k * 256: o + (k + 1) * 256] = w5[128 * k: 128 * (k + 1)]
    wt[0:18, o + 512: o + 768] = w5[256:274]
    wt[32:50, o + 512: o + 768] = w5[274:292]
    wt[64:67, o + 512: o + 768] = w5[292:295]
    set_b2(5, inputs["b5"][e])
    o = blk(8)
    wi = inputs["wi"][e]
    for k in (0, 1):
        wt[:, o + k * 256: o + (k + 1) * 256] = wi[128 * k: 128 * (k + 1)]
    set_b2(8, inputs["bi"][e])
    wa = inputs["wa"][e][:, 0]                       # [256]
    wt[:, o + 512] = wa[0:128]
    wt[:, o + 513] = wa[128:256]
    bt[0, 18 * nslot + s] = inputs["ba"][e][0]
    o = blk(10)
    wc0 = inputs["wc0"][e]                           # [283, 128]
    wt[:, o: o + 128] = wc0[0:128]
    wt[:, o + 128: o + 256] = wc0[128:256]
    wt[0:12, o + 256: o + 384] = wc0[256:268]
    wt[32:44, o + 256: o + 384] = wc0[268:280]
    wt[64:67, o + 256: o + 384] = wc0[280:283]
    bt[:, 19 * nslot + s] = inputs["bc0"][e]
    wt[:, o + 384: o + 387] = inputs["wc1"][e]
    bt[0:3, 20 * nslot + s] = inputs["bc1"][e]


def _chunk_offsets(nslot):
    coff, tot = {}, 0
    for st in CHUNKS:
        coff[st] = tot
        tot += nslot * CHUNK_COLS[st]
    return coff, tot


def _make_waves(nslot, C):
    gmax = max(1, min(512 // (2 * C), 6))
    nw = int(np.ceil(nslot / gmax))
    base = nslot // nw
    rem = nslot - base * nw
    sizes = [base + (1 if i < rem else 0) for i in range(nw)]
    waves, s0 = [], 0
    for g in sizes:
        waves.append((s0, s0 + g))
        s0 += g
    return waves


# ---------------------------------------------------------------------------
# Device program
# ---------------------------------------------------------------------------

def _build_program(C, nslot):
    """Build the SPMD Bass program: nslot expert slots of C tokens each."""
    nall = nslot * C
    waves = _make_waves(nslot, C)
    nw = len(waves)
    coff, totcols = _chunk_offsets(nslot)
    f32 = mybir.dt.float32
    bf16 = mybir.dt.bfloat16
    Sin = mybir.ActivationFunctionType.Sin
    Sigmoid = mybir.ActivationFunctionType.Sigmoid
    Relu = mybir.ActivationFunctionType.Relu
    ADD = mybir.AluOpType.add
    SUB = mybir.AluOpType.subtract
    MUL = mybir.AluOpType.mult
    MAX = mybir.AluOpType.max
    MIN = mybir.AluOpType.min
    # range-reduction constants (Cody-Waite, fp32 magic rounding)
    INV2PI = float(np.float32(1.0 / (2 * PI)))
    MAGIC = 12582912.0            # 1.5 * 2**23: forces round-to-int in fp32
    C1 = 6.28125                  # 2*pi high part, exact in fp32
    C2 = float(np.float32(2 * PI - 6.28125))
    CLAMP = 3.1415925             # just under pi (ACT Sin domain is [-pi, pi])
    HALF_PI = float(np.float32(PI / 2))

    nc = bacc.Bacc("TRN2", target_bir_lowering=False, debug=False)
    wt_d = nc.dram_tensor("wt", (128, totcols), bf16, kind="ExternalInput")
    bt_d = nc.dram_tensor("bt", (128, NB * nslot), f32, kind="ExternalInput")
    pts_d = nc.dram_tensor("ptsT", (3, nall), f32, kind="ExternalInput")
    dir_d = nc.dram_tensor("dirT", (3, nall), f32, kind="ExternalInput")
    fx_d = nc.dram_tensor("fx", (3, 18), f32, kind="ExternalInput")
    fd_d = nc.dram_tensor("fd", (3, 12), f32, kind="ExternalInput")
    al_d = nc.dram_tensor("alpha_out", (1, nall), f32, kind="ExternalOutput")
    co_d = nc.dram_tensor("color_out", (3, nall), f32, kind="ExternalOutput")

    with tile.TileContext(nc) as tc:
        with (
            tc.tile_pool(name="cp", bufs=1) as cp,
            tc.tile_pool(name="xp", bufs=2 * nw + 2) as xp,
            tc.tile_pool(name="psA", bufs=6, space=bass.MemorySpace.PSUM) as psA,
            tc.tile_pool(name="psB", bufs=2, space=bass.MemorySpace.PSUM) as psB,
        ):
            # ---- small inputs first (tiny, both HWDGE rings) ----
            embP = cp.tile([67, nall], bf16)  # points: sin 0:18, cos 32:50, xyz 64:67
            embD = cp.tile([67, nall], bf16)  # dirs:   sin 0:12, cos 32:44, xyz 64:67
            nc.vector.memset(embP[:], 0.0)
            nc.vector.memset(embD[:], 0.0)
            fx_sb = cp.tile([3, 18], f32)
            nc.sync.dma_start(fx_sb[:], fx_d.ap()[:])
            fd_sb = cp.tile([3, 12], f32)
            nc.scalar.dma_start(fd_sb[:], fd_d.ap()[:])
            pts_sb = cp.tile([3, nall], f32)
            nc.sync.dma_start(pts_sb[:], pts_d.ap()[:])
            dir_sb = cp.tile([3, nall], f32)
            nc.scalar.dma_start(dir_sb[:], dir_d.ap()[:])
            bt_sb = cp.tile([128, NB * nslot], f32)
            nc.sync.dma_start(bt_sb[:], bt_d.ap()[:])

            # ---- layer-major weight chunk DMAs, balanced over the 2 HWDGE
            # rings so the final chunks of both rings land together ----
            RING = {0: 0, 2: 0, 4: 0, 5: 0, 8: 0,     # sync:   8.52MB
                    1: 1, 3: 1, 6: 1, 7: 1, 10: 1}    # scalar: 8.09MB
            wts = {}
            for st in CHUNKS:
                wts[st] = cp.tile([128, nslot * CHUNK_COLS[st]], bf16,
                                  name=f"wt{st}", tag=f"wt{st}")
            for st in CHUNKS:
                eng = nc.sync if RING[st] == 0 else nc.scalar
                eng.dma_start(wts[st][:],
                              wt_d.ap()[:, coff[st]: coff[st] + nslot * CHUNK_COLS[st]])

            def slab(st, s, lo, hi, rows=128):
                o = s * CHUNK_COLS[st]
                return wts[st][0:rows, o + lo: o + hi]

            nc.vector.tensor_copy(embP[64:67, :], pts_sb[:])
            nc.vector.tensor_copy(embD[64:67, :], dir_sb[:])

            alpha_sb = cp.tile([1, nall], f32)
            color_sb = cp.tile([3, nall], f32)

            # frequency expansion + range-reduced sin/cos, in <=512-col chunks.
            def reduce_sin(tsrc, rows, ncol):
                t1 = xp.tile([rows, ncol], f32, tag="vred")
                nc.vector.tensor_scalar(t1[:], tsrc, INV2PI, MAGIC, MUL, ADD)
                r = xp.tile([rows, ncol], f32, tag="vred")
                nc.vector.tensor_scalar(r[:], t1[:], MAGIC, None, SUB)
                rd = xp.tile([rows, ncol], f32, tag="vred")
                nc.vector.scalar_tensor_tensor(rd[:], r[:], -C1, tsrc, MUL, ADD)
                rd2 = xp.tile([rows, ncol], f32, tag="vred")
                nc.vector.scalar_tensor_tensor(rd2[:], r[:], -C2, rd[:], MUL, ADD)
                v = xp.tile([rows, ncol], f32, tag="vred")
                nc.vector.tensor_scalar(v[:], rd2[:], CLAMP, -CLAMP, MIN, MAX)
                return v

            for lo in range(0, nall, 512):
                hi = min(nall, lo + 512)
                w_ = hi - lo
                for (rows, fmat, src, dst) in (
                    (18, fx_sb, pts_sb, embP),
                    (12, fd_sb, dir_sb, embD),
                ):
                    ep = psA.tile([rows, w_], f32, tag="mlp")
                    nc.tensor.matmul(ep[:], fmat[:, 0:rows], src[:, lo:hi],
                                     start=True, stop=True)
                    vs = reduce_sin(ep[:], rows, w_)
                    nc.scalar.activation(dst[0:rows, lo:hi], vs[:], Sin)
                    pre = xp.tile([rows, w_], f32, tag="vred")
                    nc.vector.tensor_scalar(pre[:], ep[:], HALF_PI, None, ADD)
                    vc = reduce_sin(pre[:], rows, w_)
                    cs = xp.tile([rows, w_], f32, tag="vred")
                    nc.scalar.activation(cs[:], vc[:], Sin)
                    nc.vector.tensor_copy(dst[32:32 + rows, lo:hi], cs[:])

            # ---- wave-lockstep MLP ----
            def bias2_bcast(lidx, s0, s1):
                g = s1 - s0
                ap = bt_sb[:, lidx * 2 * nslot + s0 * 2: lidx * 2 * nslot + s1 * 2]
                return ap.rearrange("p (g j) -> p j g", j=2).broadcast_to(
                    [128, 2, g, C])

            def bias1_bcast(which, s0, s1, p=128):
                g = s1 - s0
                ap = bt_sb[0:p, which * nslot + s0: which * nslot + s1]
                return ap.broadcast_to([p, g, C])

            xs = [None] * nw
            its = [None] * nw
            cts = [None] * nw

            def mm_mid(st, ps, xin, s0, s1):
                for i in range(s1 - s0):
                    s = s0 + i
                    for j in (0, 1):
                        pj = ps[:, j, i * C:(i + 1) * C]
                        nc.tensor.matmul(pj, slab(st, s, j * 128, j * 128 + 128),
                                         xin[:, 0, i * C:(i + 1) * C],
                                         start=True, stop=False)
                        nc.tensor.matmul(pj, slab(st, s, 256 + j * 128, 256 + j * 128 + 128),
                                         xin[:, 1, i * C:(i + 1) * C],
                                         start=False, stop=True)

            def move2(ps, lidx, s0, s1, relu=True):
                g = s1 - s0
                xn = xp.tile([128, 2, g * C], bf16, tag="x")
                psv = ps[:].rearrange("p j (g c) -> p j g c", g=g)
                xnv = xn[:].rearrange("p j (g c) -> p j g c", g=g)
                nc.vector.tensor_tensor(xnv, psv, bias2_bcast(lidx, s0, s1), ADD)
                if relu:
                    nc.scalar.activation(xn[:], xn[:], Relu)
                return xn

            def emit_stage(wi_, stage):
                s0, s1 = waves[wi_]
                g = s1 - s0
                if stage == 0:  # L0
                    ps = psA.tile([128, 2, g * C], f32, tag="mlp")
                    for i in range(g):
                        s = s0 + i
                        sl = slice(s * C, (s + 1) * C)
                        for j in (0, 1):
                            nc.tensor.matmul(ps[:, j, i * C:(i + 1) * C],
                                             slab(0, s, j * 128, j * 128 + 128, rows=67),
                                             embP[0:67, sl], start=True, stop=True)
                    xs[wi_] = move2(ps, 0, s0, s1)
                elif stage in (1, 2, 3, 4, 6, 7):
                    ps = psA.tile([128, 2, g * C], f32, tag="mlp")
                    mm_mid(stage, ps, xs[wi_], s0, s1)
                    xs[wi_] = move2(ps, stage, s0, s1)
                elif stage == 5:
                    ps = psA.tile([128, 2, g * C], f32, tag="mlp")
                    xin = xs[wi_]
                    for i in range(g):
                        s = s0 + i
                        sl = slice(s * C, (s + 1) * C)
                        for j in (0, 1):
                            pj = ps[:, j, i * C:(i + 1) * C]
                            nc.tensor.matmul(pj, slab(5, s, j * 128, j * 128 + 128),
                                             xin[:, 0, i * C:(i + 1) * C],
                                             start=True, stop=False)
                            nc.tensor.matmul(pj, slab(5, s, 256 + j * 128, 256 + j * 128 + 128),
                                             xin[:, 1, i * C:(i + 1) * C],
                                             start=False, stop=False)
                            nc.tensor.matmul(pj, slab(5, s, 512 + j * 128, 512 + j * 128 + 128, rows=67),
                                             embP[0:67, sl], start=False, stop=True)
                    xs[wi_] = move2(ps, 5, s0, s1)
                elif stage == 8:  # wi -> inter (bias, no relu)
                    ps = psA.tile([128, 2, g * C], f32, tag="mlp")
                    mm_mid(8, ps, xs[wi_], s0, s1)
                    its[wi_] = move2(ps, 8, s0, s1, relu=False)
                elif stage == 9:  # wa -> alpha (weights ride in chunk 8)
                    pa = psB.tile([3, g * C], f32, tag="head")
                    xin = xs[wi_]
                    for i in range(g):
                        s = s0 + i
                        nc.tensor.matmul(pa[0:1, i * C:(i + 1) * C],
                                         slab(8, s, 512, 513),
                                         xin[:, 0, i * C:(i + 1) * C],
                                         start=True, stop=False)
                        nc.tensor.matmul(pa[0:1, i * C:(i + 1) * C],
                                         slab(8, s, 513, 514),
                                         xin[:, 1, i * C:(i + 1) * C],
                                         start=False, stop=True)
                    av = alpha_sb[0:1, s0 * C: s1 * C].rearrange(
                        "p (g c) -> p g c", g=g)
                    pav = pa[0:1, :].rearrange("p (g c) -> p g c", g=g)
                    nc.vector.tensor_tensor(av, pav, bias1_bcast(18, s0, s1, p=1), ADD)
                elif stage == 10:  # wc0 -> c (relu)
                    pc = psA.tile([128, g * C], f32, tag="mlp")
                    it = its[wi_]
                    for i in range(g):
                        s = s0 + i
                        sl = slice(s * C, (s + 1) * C)
                        pj = pc[:, i * C:(i + 1) * C]
                        nc.tensor.matmul(pj, slab(10, s, 0, 128),
                                         it[:, 0, i * C:(i + 1) * C],
                                         start=True, stop=False)
                        nc.tensor.matmul(pj, slab(10, s, 128, 256),
                                         it[:, 1, i * C:(i + 1) * C],
                                         start=False, stop=False)
                        nc.tensor.matmul(pj, slab(10, s, 256, 384, rows=67),
                                         embD[0:67, sl], start=False, stop=True)
                    ct = xp.tile([128, g * C], bf16, tag="ct")
                    pcv = pc[:].rearrange("p (g c) -> p g c", g=g)
                    ctv = ct[:].rearrange("p (g c) -> p g c", g=g)
                    nc.vector.tensor_tensor(ctv, pcv, bias1_bcast(19, s0, s1), ADD)
                    nc.scalar.activation(ct[:], ct[:], Relu)
                    cts[wi_] = ct
                elif stage == 11:  # wc1 -> sigmoid color (weights in chunk 10)
                    pcol = psB.tile([3, g * C], f32, tag="head")
                    ct = cts[wi_]
                    for i in range(g):
                        s = s0 + i
                        nc.tensor.matmul(pcol[:, i * C:(i + 1) * C],
                                         slab(10, s, 384, 387),
                                         ct[:, i * C:(i + 1) * C],
                                         start=True, stop=True)
                    ctmp = xp.tile([3, g * C], f32, tag="ctmp")
                    pv = pcol[:].rearrange("p (g c) -> p g c", g=g)
                    cv = ctmp[:].rearrange("p (g c) -> p g c", g=g)
                    nc.vector.tensor_tensor(cv, pv, bias1_bcast(20, s0, s1, p=3), ADD)
                    nc.scalar.activation(color_sb[0:3, s0 * C: s1 * C], ctmp[:],
                                         Sigmoid)

            for stage in range(12):
                for wi_ in range(nw):
                    emit_stage(wi_, stage)

            nc.sync.dma_start(al_d.ap()[:], alpha_sb[:])
            nc.sync.dma_start(co_d.ap()[:], color_sb[:])

    nc.compile()
    return nc


_prog_cache = {}
_last_results = None


def _get_program(C, nslot):
    key = (C, nslot)
    if key not in _prog_cache:
        _prog_cache[key] = _build_program(C, nslot)
    return _prog_cache[key]


# ---------------------------------------------------------------------------
# Host wrapper
# ---------------------------------------------------------------------------

def kernel(**inputs):
    global _last_results
    inputs = {k: np.asarray(v) for k, v in inputs.items()}
    idx = inputs["index"].astype(np.int64)
    B = idx.shape[0]
    points = inputs["points"].astype(np.float32)
    dirs = inputs["directions"].astype(np.float32)

    # --- routing: split each expert's tokens into <=CAP_MAX chunks ("virtual
    # experts"), distribute round-robin (sorted by size) over 8 cores ---
    tok = [np.nonzero(idx == e)[0] for e in range(E)]
    virt = []  # (expert, token_ids)
    for e in range(E):
        t = tok[e]
        if len(t) == 0:
            continue
        for lo in range(0, len(t), CAP_MAX):
            virt.append((e, t[lo: lo + CAP_MAX]))
    if not virt:
        virt = [(0, np.zeros((0,), np.int64))]
    virt.sort(key=lambda v: -len(v[1]))
    nslot = max(1, int(np.ceil(len(virt) / N_CORES)))
    C = max(4, int(np.ceil(max(len(v[1]) for v in virt) / 4) * 4))
    nall = nslot * C

    core_slots = [[] for _ in range(N_CORES)]
    for i, v in enumerate(virt):
        core_slots[i % N_CORES].append(v)

    nc = _get_program(C, nslot)
    coff, totcols = _chunk_offsets(nslot)

    fx = np.zeros((3, 18), np.float32)
    for c in range(3):
        for k in range(NX):
            fx[c, c * NX + k] = float(2 ** k)
    fd = np.zeros((3, 12), np.float32)
    for c in range(3):
        for k in range(ND):
            fd[c, c * ND + k] = float(2 ** k)

    in_maps = []
    for c in range(N_CORES):
        wt = np.zeros((128, totcols), np.float32)
        bt = np.zeros((128, NB * nslot), np.float32)
        ptsT = np.zeros((3, nall), np.float32)
        dirT = np.zeros((3, nall), np.float32)
        for s, (e, t) in enumerate(core_slots[c]):
            _pack_expert(wt, bt, s, nslot, inputs, e, coff)
            n = len(t)
            if n:
                ptsT[:, s * C: s * C + n] = points[t].T
                dirT[:, s * C: s * C + n] = dirs[t].T
        in_maps.append({"wt": wt.astype(ml_dtypes.bfloat16), "bt": bt,
                        "ptsT": ptsT, "dirT": dirT, "fx": fx, "fd": fd})

    res = run_bass_kernel_spmd(nc, in_maps, core_ids=list(range(N_CORES)))
    _last_results = res

    out = np.zeros((B, 4), np.float32)
    for c in range(N_CORES):
        al = res.results[c]["alpha_out"]
        co = res.results[c]["color_out"]
        for s, (e, t) in enumerate(core_slots[c]):
            n = len(t)
            if n:
                out[t, 0] = al[0, s * C: s * C + n]
                out[t, 1:4] = co[:, s * C: s * C + n].T
    return out
